# revision 1
# baseline (speedup 1.0000x reference)
"""Trainium2 Bass kernel for nn_AttentionBlock (sparse_attention, no-softmax).

Computation (per batch b):
    qh = (q @ Wq^T) split into 16 heads of dk=64     [S, D] -> [H, S, DK]
    kh, vh likewise
    scores = (qh @ kh^T) / sqrt(DK)                  [H, S, S]
    p      = scores * A^T                            (elementwise structural mask)
    x      = p @ vh                                  [H, S, DK] -> [S, D]
    out    = x @ Wo^T + bo                           [S, D]

Sharding over 8 NeuronCores: data-parallel over batch (B=2) x tensor-parallel
over heads (16 heads -> 4 per core). Each core projects q/k/v for its 4 heads
(column-parallel), runs masked attention for them, and applies its 256-column
slice of the output projection (row-parallel), producing a full-shape partial
output. Host sums the 4 partials per batch.

Implementation notes:
- Activations are shipped pre-transposed ([D, S]) so every matmul contraction
  dim lands on SBUF partitions with no on-device transposes; 1/sqrt(DK) is
  folded into the mask A on the host.
- The whole data path runs in fp16 with fp32 PSUM accumulation (all operands
  here are O(1)-O(100), well inside fp16 range; measured end-to-end error is
  a few 1e-4). fp16 is the same PE stream rate as bf16/f32r but, being
  2-byte, additionally halves DMA/SBUF traffic and legalizes PE quadrant
  packing (tile_position), which f32/f32r reject.
- Heads are stored as pairs on the partition axis (head 2j on partitions
  0:63, head 2j+1 on 64:127). The K=64 score matmuls of a pair run
  concurrently in the upper/lower PE row-quadrants (tile_position (0,0) /
  (64,0)); the M=64 p@v matmuls of a pair run concurrently in left/right
  col-quadrants into one PSUM bank (tile_position (0,0) / (0,64)).
- The mask multiply is the throughput-critical elementwise stage; it is
  spread over three engines per the MASK_ROUTE pattern: DVE straight out of
  PSUM (most tiles), a ScalarE PSUM->SBUF bounce feeding GPSIMD (head 3 +
  head 2 every 8th key block), and occasionally a ScalarE f16 bounce feeding
  a DVE 2x all-SBUF multiply. GPSIMD cannot read PSUM (neuronxcc rejects
  it), so its tiles must bounce through ScalarE.
- Projection work for the next/previous query block is interleaved into the
  attention loop so no engine drains the pipeline at block boundaries.
  The prologue orders DMAs by true need time (k weights/activations, then
  q, then v, then the mask tiles); slice-0 v-chains drain as group-0
  fillers so scores start ~13us in. Partial outputs and the o-projection
  weights are fp16 (halves outbound DMA; host accumulates in fp32), and
  the epilogue alternates its PSUM->SBUF drains between ScalarE and DVE.
"""

import numpy as np

import concourse.mybir as mybir
import concourse.tile as tile
from concourse import bacc, bass_utils
from concourse.bass import AP


def _bcast_mid(ap2, n):
    """[128, F] AP -> [128, (0-stride n), F]: broadcast over an inserted
    middle dim so one tensor_tensor applies the same mask row-block to n
    head slots."""
    lay = [list(d) for d in ap2.ap]
    assert len(lay) == 2
    return AP(ap2.tensor, ap2.offset, [lay[0], [0, n], lay[1]])

B, S, D, H = 2, 2048, 1024, 16
NCORES = 8
GROUPS = NCORES // B          # 4 head-groups
HPC = H // GROUPS             # 4 heads per core
DK = D // H                   # 64
HD = HPC * DK                 # 256 head-dim columns per core
NPAIR = HPC // 2              # 2 head pairs per core
SCALE = 1.0 / np.sqrt(DK)

P = 128                       # SBUF partitions
QB = 512                      # query block
NQB = S // QB                 # 4
KBLK = 128                    # key block
NKB = S // KBLK               # 16
NKT = D // P                  # 8 contraction chunks for projections
AGRP = 4                      # key-blocks per A-tile DMA / interleave group
NGRP = NKB // AGRP            # 4 groups

f32 = mybir.dt.float32
f16 = mybir.dt.float16

import os
KV_SPLIT = int(os.environ.get("KV_SPLIT", "1"))       # split k/q chains in half
MERGE_KQ = int(os.environ.get("MERGE_KQ", "1"))       # merged khT/qhT drains
MERGE_XTS = int(os.environ.get("MERGE_XTS", "0"))     # merged xts drain
OPROJ_SPLIT = int(os.environ.get("OPROJ_SPLIT", "1")) # o-proj per-et thunks
A_PREF = int(os.environ.get("A_PREF", "1"))           # A prefetch depth (1|2)
GP_MOD = int(os.environ.get("GP_MOD", "8"))           # head2 on GP when kb%GP_MOD==0
OUT_F16 = int(os.environ.get("OUT_F16", "1"))         # fp16 partial outputs
DVE_MERGE = int(os.environ.get("DVE_MERGE", "0"))     # heads 0-2 in one DVE mask op
SCG_BUFS = int(os.environ.get("SCG_BUFS", "1"))       # psum bufs for the GP head's scores
U_BUFS = int(os.environ.get("U_BUFS", "2"))           # psum bufs for proj chains
SC_BUFS = int(os.environ.get("SC_BUFS", "4"))         # psum bufs for score tiles
BOUNCE_HALF = int(os.environ.get("BOUNCE_HALF", "0")) # split ScalarE->GPSIMD path in halves
GP_PSUM = int(os.environ.get("GP_PSUM", "0"))         # GPSIMD reads scores straight from PSUM
OPROJ_LATE = int(os.environ.get("OPROJ_LATE", "1"))   # last qb: prev o-proj in late groups
ACT_ASSIST = int(os.environ.get("ACT_ASSIST", "0"))   # h2 via Act f16-bounce + DVE 4x mult
MASK_ROUTE = os.environ.get("MASK_ROUTE", "ddggdddgddggddagddggdddgddggddagdddgddggddagddggdddgddggddagdddg")         # per-head route: d=DVE, a=Act+DVE2x, g=GPSIMD
WO_F16 = int(os.environ.get("WO_F16", "1"))           # f16 output projection weights + xts
WO_DEFER = int(os.environ.get("WO_DEFER", "1"))       # load wo during qb0 (not prologue)
EPI_DVE = int(os.environ.get("EPI_DVE", "1"))         # epilogue osb copies alternate DVE/Act
GP_MOD1 = int(os.environ.get("GP_MOD1", "0"))         # GP_MOD override for qb>=1 (0=same)
EPI_DMA_SPLIT = int(os.environ.get("EPI_DMA_SPLIT", "0"))  # epilogue DMA per 512-col chunk
PRO_QFIRST = int(os.environ.get("PRO_QFIRST", "1"))   # prologue: q proj before v proj
V_FILLER = int(os.environ.get("V_FILLER", "1"))       # slice-0 v chains as group-0 fillers
A_EARLY = int(os.environ.get("A_EARLY", "0"))
A_AFTER_KV = int(os.environ.get("A_AFTER_KV", "0"))   # group A-prefetch after kv DMAs         # A0/A1 before the v stream
PT_BUFS = int(os.environ.get("PT_BUFS", "32"))        # sbuf bufs for mask outputs
SCB_BUFS = int(os.environ.get("SCB_BUFS", "12"))       # sbuf bufs for GP bounce tiles
SC_PAIR = int(os.environ.get("SC_PAIR", "0"))         # heads 0+1 share a 2-bank score tile
PRO_SPLIT2 = int(os.environ.get("PRO_SPLIT2", "0"))   # finer first wk/kT0 DMA chunks
QB0_PLAIN = int(os.environ.get("QB0_PLAIN", "0"))     # no Act-assist routing in qb0
PV_DEPTH = int(os.environ.get("PV_DEPTH", "5"))       # p@v software-pipeline depth (key blocks)
QT_SPLIT = int(os.environ.get("QT_SPLIT", "0"))       # split qT loads in kt halves
FILL_RATE = int(os.environ.get("FILL_RATE", "2"))     # filler thunks drained per key block
PV_POS = int(os.environ.get("PV_POS", "0"))           # emit pipelined p@v before the masks
OPROJ_SPREAD = int(os.environ.get("OPROJ_SPREAD", "0"))  # 1 o-proj chain per group (not 2+2)
                                                      # and one merged DVE mask multiply
                                                      # e.g. "daag"; empty = legacy GP_MOD/ACT_ASSIST

_CACHED = None  # built module, reused across kernel() calls
TRACE = False         # set True (e.g. from test.py) to profile the NEFF
LAST_RESULTS = None   # BassKernelResults of the most recent run


def _build():
    nc = bacc.Bacc("TRN2", target_bir_lowering=False)

    qT = nc.dram_tensor("qT", [D, S], f16, kind="ExternalInput")
    kT = nc.dram_tensor("kT", [D, S], f16, kind="ExternalInput")
    vT = nc.dram_tensor("vT", [D, S], f16, kind="ExternalInput")
    Asc = nc.dram_tensor("Asc", [S, S], f16, kind="ExternalInput")
    wq = nc.dram_tensor("wq", [D, HD], f16, kind="ExternalInput")
    wk = nc.dram_tensor("wk", [D, HD], f16, kind="ExternalInput")
    wv = nc.dram_tensor("wv", [D, HD], f16, kind="ExternalInput")
    wo = nc.dram_tensor("wo", [HD, D],
                        f16 if WO_F16 else mybir.dt.float32r, kind="ExternalInput")
    # fp16 partial outputs: halves the outbound DMA; the host accumulates the
    # four per-batch partials in fp32 (adds ~1e-4 relative error)
    out = nc.dram_tensor("out", [S, D], f16 if OUT_F16 else f32, kind="ExternalOutput")

    qT_r = qT.rearrange("(kt p) s -> p kt s", p=P)
    kT_r = kT.rearrange("(kt p) s -> p kt s", p=P)
    vT_r = vT.rearrange("(kt p) s -> p kt s", p=P)
    wq_r = wq.rearrange("(kt p) c -> p kt c", p=P)
    wk_r = wk.rearrange("(kt p) c -> p kt c", p=P)
    wv_r = wv.rearrange("(kt p) c -> p kt c", p=P)
    wo_r = wo.rearrange("(ck p) e -> p ck e", p=P)
    A_r = Asc.rearrange("(kb p) q -> p kb q", p=P)

    with tile.TileContext(nc) as tc:
        with (
            tc.tile_pool(name="persist", bufs=1) as pp,
            tc.tile_pool(name="stream", bufs=2) as sp,
            tc.tile_pool(name="psU", bufs=2, space="PSUM") as psU,   # proj [128,512] x2 + sc x4
            tc.tile_pool(name="psX", bufs=1, space="PSUM") as psX,   # xT accumulators
        ):
            wk_sb = pp.tile([P, NKT, HD], f16, tag="wk")
            wv_sb = pp.tile([P, NKT, HD], f16, tag="wv")
            wq_sb = pp.tile([P, NKT, HD], f16, tag="wq")
            wo_sb = pp.tile([P, HD // P, D],
                            f16 if WO_F16 else mybir.dt.float32r, tag="wo")

            # head-PAIR layout: pair j holds head 2j on partitions 0:64 and
            # head 2j+1 on 64:128 — the layout quadrant packing requires
            khT_sb = pp.tile([P, NPAIR, S], f16, tag="khT")
            vh_sb = pp.tile([P, NKB, HD], f16, tag="vh")     # [ks%128, kb, c]

            def kv_proj_chains(st, kT_pre=None, vT_pre=None):
                """Issue k/v DMAs for s-slice st now; return per-chain thunks
                so the projection matmuls can be sprinkled between attention
                iterations instead of lumped at group boundaries."""
                sl = slice(st * QB, (st + 1) * QB)
                kT_sb = kT_pre
                if kT_sb is None:
                    kT_sb = sp.tile([P, NKT, QB], f16, tag="xin", bufs=4, name="kT_sb")
                    nc.sync.dma_start(kT_sb[:], kT_r[:, :, sl])
                vT_sb = vT_pre
                if vT_sb is None:
                    vT_sb = sp.tile([P, NKT, QB], f16, tag="xin", bufs=4, name="vT_sb")
                    nc.sync.dma_start(vT_sb[:], vT_r[:, :, sl])

                pk_tiles = {}

                def kchain_a(ct, kT_sb=kT_sb):
                    pk = psU.tile([P, QB], f32, tag="u", bufs=U_BUFS, name="pk")
                    pk_tiles[ct] = pk
                    hi = NKT // 2 if KV_SPLIT else NKT
                    for kt in range(hi):
                        nc.tensor.matmul(
                            pk[:], wk_sb[:, kt, ct * P:(ct + 1) * P], kT_sb[:, kt, :],
                            start=(kt == 0), stop=(kt == NKT - 1),
                        )
                    if not KV_SPLIT:
                        kdrain(ct)

                def kdrain(ct):
                    pk = pk_tiles.pop(ct)
                    if MERGE_KQ:
                        nc.scalar.copy(khT_sb[:, ct, sl], pk[:])
                    else:
                        nc.scalar.copy(khT_sb[0:DK, ct, sl], pk[0:DK, :])
                        nc.scalar.copy(khT_sb[DK:P, ct, sl], pk[DK:P, :])

                def kchain_b(ct, kT_sb=kT_sb):
                    pk = pk_tiles[ct]
                    for kt in range(NKT // 2, NKT):
                        nc.tensor.matmul(
                            pk[:], wk_sb[:, kt, ct * P:(ct + 1) * P], kT_sb[:, kt, :],
                            start=False, stop=(kt == NKT - 1),
                        )
                    kdrain(ct)

                def vchain(ssub, vT_sb=vT_sb):
                    kb = st * (QB // P) + ssub
                    pv = psU.tile([P, HD], f32, tag="u", bufs=U_BUFS, name="pv")
                    for kt in range(NKT):
                        nc.tensor.matmul(
                            pv[:], vT_sb[:, kt, ssub * P:(ssub + 1) * P], wv_sb[:, kt, :],
                            start=(kt == 0), stop=(kt == NKT - 1),
                        )
                    nc.scalar.copy(vh_sb[:, kb, :], pv[:])

                thunks = []
                for ct in range(NPAIR):
                    thunks.append(lambda ct=ct: kchain_a(ct))
                    if KV_SPLIT:
                        thunks.append(lambda ct=ct: kchain_b(ct))
                thunks += [lambda s=s: vchain(s) for s in range(QB // P)]
                return thunks

            def kv_proj(st, kT_pre=None, vT_pre=None):
                for thunk in kv_proj_chains(st, kT_pre, vT_pre):
                    thunk()

            def q_proj_chains(qb):
                """Issue the q DMA now; return (qhT tile, per-pair chain thunks)."""
                qsl = slice(qb * QB, (qb + 1) * QB)
                qT_sb = sp.tile([P, NKT, QB], f16, tag="xin", bufs=4, name="qT_sb")
                if QT_SPLIT:
                    nc.sync.dma_start(qT_sb[:, 0:NKT // 2, :], qT_r[:, 0:NKT // 2, qsl])
                    nc.sync.dma_start(qT_sb[:, NKT // 2:, :], qT_r[:, NKT // 2:, qsl])
                else:
                    nc.sync.dma_start(qT_sb[:], qT_r[:, :, qsl])
                qhT_sb = sp.tile([P, NPAIR, QB], f16, tag="qh", bufs=3, name="qhT_sb")

                pq_tiles = {}

                def chain_a(ct):
                    pq = psU.tile([P, QB], f32, tag="u", bufs=U_BUFS, name="pq")
                    pq_tiles[ct] = pq
                    hi = NKT // 2 if KV_SPLIT else NKT
                    for kt in range(hi):
                        nc.tensor.matmul(
                            pq[:], wq_sb[:, kt, ct * P:(ct + 1) * P], qT_sb[:, kt, :],
                            start=(kt == 0), stop=(kt == NKT - 1),
                        )
                    if not KV_SPLIT:
                        qdrain(ct)

                def qdrain(ct):
                    pq = pq_tiles.pop(ct)
                    if MERGE_KQ:
                        nc.scalar.copy(qhT_sb[:, ct, :], pq[:])
                    else:
                        nc.scalar.copy(qhT_sb[0:DK, ct, :], pq[0:DK, :])
                        nc.scalar.copy(qhT_sb[DK:P, ct, :], pq[DK:P, :])

                def chain_b(ct):
                    pq = pq_tiles[ct]
                    for kt in range(NKT // 2, NKT):
                        nc.tensor.matmul(
                            pq[:], wq_sb[:, kt, ct * P:(ct + 1) * P], qT_sb[:, kt, :],
                            start=False, stop=(kt == NKT - 1),
                        )
                    qdrain(ct)

                thunks = []
                for ct in range(NPAIR):
                    thunks.append(lambda ct=ct: chain_a(ct))
                    if KV_SPLIT:
                        thunks.append(lambda ct=ct: chain_b(ct))
                return qhT_sb, thunks

            def q_proj(qb):
                qhT_sb, chains = q_proj_chains(qb)
                for thunk in chains:
                    thunk()
                return qhT_sb

            def o_proj_et(xts, qb, ssub, et, osb, ptag="u", pbufs=None,
                          dve_copy=False, epi_split=False):
                """One 512-col chunk of the output projection for query block
                qb, row-slice ssub; et==last also issues the out DMA."""
                if pbufs is None and ptag == "u":
                    pbufs = U_BUFS
                po = psU.tile([P, QB], f32, tag=ptag, bufs=pbufs, name="po")
                for ck in range(HD // P):
                    nc.tensor.matmul(
                        po[:],
                        xts[:, ck, ssub * P:(ssub + 1) * P],
                        wo_sb[:, ck, et * QB:(et + 1) * QB],
                        start=(ck == 0), stop=(ck == HD // P - 1),
                    )
                if dve_copy:
                    nc.vector.tensor_copy(osb[:, et * QB:(et + 1) * QB], po[:])
                else:
                    nc.scalar.copy(osb[:, et * QB:(et + 1) * QB], po[:])
                rsl = slice(qb * QB + ssub * P, qb * QB + (ssub + 1) * P)
                if epi_split:
                    nc.sync.dma_start(
                        out[rsl, et * QB:(et + 1) * QB],
                        osb[:, et * QB:(et + 1) * QB],
                    )
                elif et == D // QB - 1:
                    nc.sync.dma_start(out[rsl, :], osb[:])

            def o_proj_chain_thunks(xts, qb, ssub, ptag="u", pbufs=None,
                                    epi=False):
                osb = sp.tile([P, D], f16 if OUT_F16 else f32, tag="osb", bufs=4, name="osb")
                ets = [
                    lambda et=et: o_proj_et(xts, qb, ssub, et, osb, ptag, pbufs,
                                            dve_copy=epi and EPI_DVE and et % 2 == 0,
                                            epi_split=epi and EPI_DMA_SPLIT)
                    for et in range(D // QB)
                ]
                if OPROJ_SPLIT:
                    return ets
                def whole():
                    for t in ets:
                        t()
                return [whole]

            def o_proj_chain(xts, qb, ssub, ptag="u", pbufs=None, epi=False):
                for thunk in o_proj_chain_thunks(xts, qb, ssub, ptag, pbufs, epi):
                    thunk()

            # ---- pipeline ---------------------------------------------------------
            # prologue DMAs in dependency-first order so the PE starts ASAP
            kT0 = sp.tile([P, NKT, QB], f16, tag="xin", bufs=4, name="kT_sb")
            if PRO_SPLIT2:
                nc.sync.dma_start(wk_sb[:, 0:2, :], wk_r[:, 0:2, :])
                nc.sync.dma_start(kT0[:, 0:2, :], kT_r[:, 0:2, 0:QB])
                nc.sync.dma_start(wk_sb[:, 2:NKT // 2, :], wk_r[:, 2:NKT // 2, :])
                nc.sync.dma_start(kT0[:, 2:NKT // 2, :], kT_r[:, 2:NKT // 2, 0:QB])
            else:
                nc.sync.dma_start(wk_sb[:, 0:NKT // 2, :], wk_r[:, 0:NKT // 2, :])
                nc.sync.dma_start(kT0[:, 0:NKT // 2, :], kT_r[:, 0:NKT // 2, 0:QB])
            nc.sync.dma_start(wk_sb[:, NKT // 2:, :], wk_r[:, NKT // 2:, :])
            nc.sync.dma_start(kT0[:, NKT // 2:, :], kT_r[:, NKT // 2:, 0:QB])
            if not PRO_QFIRST:
                nc.sync.dma_start(wv_sb[:], wv_r[:])
            vT0 = sp.tile([P, NKT, QB], f16, tag="xin", bufs=4, name="vT_sb")
            if not PRO_QFIRST:
                nc.sync.dma_start(vT0[:], vT_r[:, :, 0:QB])
            nc.sync.dma_start(wq_sb[:], wq_r[:])

            # A tiles are fetched two groups ahead; group sequence is linear
            # over (qb, kbg)
            gseq = [(qb, kbg) for qb in range(NQB) for kbg in range(NGRP)]
            A_tiles = {}

            def fetch_A(gidx):
                if gidx >= len(gseq):
                    return
                aqb, akbg = gseq[gidx]
                A_sb = sp.tile([P, AGRP, QB], f16, tag="A", bufs=4, name="A_sb")
                nc.sync.dma_start(
                    A_sb[:],
                    A_r[:, akbg * AGRP:(akbg + 1) * AGRP, aqb * QB:(aqb + 1) * QB],
                )
                A_tiles[gidx] = A_sb

            pend_fillers = []
            if PRO_QFIRST:
                # q projection (and its DMA) before the v stream: scores can
                # start ~6us earlier; v data is not needed until the first p@v
                qhT_cur, qchains0 = q_proj_chains(0)
                if A_EARLY:
                    fetch_A(0)
                    if A_PREF >= 2:
                        fetch_A(1)
                nc.sync.dma_start(wv_sb[:], wv_r[:])
                nc.sync.dma_start(vT0[:], vT_r[:, :, 0:QB])
                if not A_EARLY:
                    fetch_A(0)
                    if A_PREF >= 2:
                        fetch_A(1)
                thunks0 = kv_proj_chains(0, kT_pre=kT0, vT_pre=vT0)
                for t in thunks0[:2 * NPAIR]:   # k chains
                    t()
                for t in qchains0:              # q chains
                    t()
                if V_FILLER:
                    pend_fillers = thunks0[2 * NPAIR:]
                else:
                    for t in thunks0[2 * NPAIR:]:   # v chains
                        t()
            else:
                fetch_A(0)
                kv_proj(0, kT_pre=kT0, vT_pre=vT0)
                qhT_cur = q_proj(0)
                if A_PREF >= 2:
                    fetch_A(1)
            if not WO_DEFER:
                nc.sync.dma_start(wo_sb[:], wo_r[:])

            pend_xts = None    # (xts tile, qb) awaiting output projection
            qhT_next = None

            for qb in range(NQB):
                qsl = slice(qb * QB, (qb + 1) * QB)
                xt = psX.tile([P, NPAIR, QB], f32, tag="xt", name="xt")  # 2 banks
                xts = sp.tile([P, NPAIR, QB],
                              f16 if WO_F16 else mybir.dt.float32r,
                              tag="xts", bufs=3, name="xts")
                pend_pts = []

                def emit_xt(pts, kb, xt=xt, xts=xts):
                    # p @ v: both heads of a pair run concurrently in the
                    # left/right PE col-quadrants into one PSUM bank.
                    # skip_group_check: the two col-quadrant groups legally
                    # share one PSUM bank (sim-only guard).
                    for j in range(NPAIR):
                        nc.tensor.matmul(
                            xt[0:DK, j, :],
                            vh_sb[:, kb, (2 * j) * DK:(2 * j + 1) * DK],
                            pts[2 * j],
                            start=(kb == 0), stop=(kb == NKB - 1),
                            tile_position=(0, 0), skip_group_check=True,
                        )
                        nc.tensor.matmul(
                            xt[DK:P, j, :],
                            vh_sb[:, kb, (2 * j + 1) * DK:(2 * j + 2) * DK],
                            pts[2 * j + 1],
                            start=(kb == 0), stop=(kb == NKB - 1),
                            tile_position=(0, DK), skip_group_check=True,
                        )
                    if kb == NKB - 1:
                        if MERGE_XTS:
                            nc.scalar.copy(xts[:], xt[:])
                        else:
                            nc.scalar.copy(xts[:, 0, :], xt[:, 0, :])
                            nc.scalar.copy(xts[:, 1, :], xt[:, 1, :])

                fillers = list(pend_fillers)
                pend_fillers = []
                for kbg in range(NGRP):
                    gidx = qb * NGRP + kbg
                    if gidx not in A_tiles:
                        fetch_A(gidx)
                    A_sb = A_tiles.pop(gidx)
                    if not A_AFTER_KV:
                        fetch_A(gidx + A_PREF)
                    # queue this group's independent projection work; it is
                    # drained two chains per key-block below, keeping the PE
                    # fed without starving the elementwise engines
                    if qb == 0:
                        if kbg == 2 and WO_DEFER:
                            nc.sync.dma_start(wo_sb[:], wo_r[:])
                        if kbg < NGRP - 1:
                            fillers += kv_proj_chains(kbg + 1)
                        else:
                            qhT_next, qchains = q_proj_chains(1)
                            fillers += qchains
                    else:
                        late = OPROJ_LATE and qb == NQB - 1
                        if OPROJ_SPREAD:
                            og = kbg if not late else kbg - 1
                            lastog = NGRP - 1 if not late else NGRP - 2
                        else:
                            og = kbg - 2 if late else kbg
                            lastog = 1
                        if 0 <= og <= lastog and pend_xts is not None:
                            xts_p, qb_p = pend_xts
                            if OPROJ_SPREAD:
                                fillers += o_proj_chain_thunks(xts_p, qb_p, og)
                            else:
                                fillers += o_proj_chain_thunks(xts_p, qb_p, 2 * og)
                                fillers += o_proj_chain_thunks(xts_p, qb_p, 2 * og + 1)
                            if og == lastog:
                                pend_xts = None
                        if kbg == NGRP - 1 and qb < NQB - 1:
                            qhT_next, qchains = q_proj_chains(qb + 1)
                            fillers += qchains
                    if A_AFTER_KV:
                        fetch_A(gidx + A_PREF)
                    for i in range(AGRP):
                        kb = kbg * AGRP + i
                        ksl = slice(kb * KBLK, (kb + 1) * KBLK)
                        if SC_PAIR:
                            # heads 0,1: quadrant-packed score pair into one
                            # 2-bank tile; both banks finish together, so one
                            # merged DVE multiply adds no latency
                            sc2 = psU.tile([P, 2, QB], f32, tag="sc2", bufs=1,
                                           name="sc2")
                            sch2 = psU.tile([P, QB], f32, tag="sch2", bufs=1,
                                            name="sch2")
                            sch3 = psU.tile([P, QB], f32, tag="sch3", bufs=1,
                                            name="sch3")
                            nc.tensor.matmul(
                                sc2[:, 0, :], khT_sb[0:DK, 0, ksl], qhT_cur[0:DK, 0, :],
                                start=True, stop=True, tile_position=(0, 0),
                                skip_group_check=True,
                            )
                            nc.tensor.matmul(
                                sc2[:, 1, :], khT_sb[DK:P, 0, ksl], qhT_cur[DK:P, 0, :],
                                start=True, stop=True, tile_position=(DK, 0),
                                skip_group_check=True,
                            )
                            nc.tensor.matmul(
                                sch2[:], khT_sb[0:DK, 1, ksl], qhT_cur[0:DK, 1, :],
                                start=True, stop=True, tile_position=(0, 0),
                                skip_group_check=True,
                            )
                            nc.tensor.matmul(
                                sch3[:], khT_sb[DK:P, 1, ksl], qhT_cur[DK:P, 1, :],
                                start=True, stop=True, tile_position=(DK, 0),
                                skip_group_check=True,
                            )
                            pt2 = sp.tile([P, 2, QB], f16, tag="pt2", bufs=3,
                                          name="pt2")
                            nc.vector.tensor_tensor(
                                pt2[:], sc2[:], _bcast_mid(A_sb[:, i, :], 2),
                                mybir.AluOpType.mult,
                            )
                            pts = [pt2[:, 0, :], pt2[:, 1, :]]
                            for h, sch in ((2, sch2), (3, sch3)):
                                pt = sp.tile([P, QB], f16, tag="pt", bufs=PT_BUFS,
                                             name="pt")
                                r = MASK_ROUTE[(kb * HPC + h) % len(MASK_ROUTE)] \
                                    if len(MASK_ROUTE) > HPC else MASK_ROUTE[h]
                                if r == "g":
                                    sc_sb = sp.tile([P, QB], f32, tag="scb",
                                                    bufs=SCB_BUFS, name="sc_sb")
                                    nc.scalar.copy(sc_sb[:], sch[:])
                                    nc.gpsimd.tensor_tensor(
                                        pt[:], sc_sb[:], A_sb[:, i, :],
                                        mybir.AluOpType.mult,
                                    )
                                elif r == "a":
                                    sc_sb = sp.tile([P, QB], f16, tag="scbh", bufs=4,
                                                    name="sc_sbh")
                                    nc.scalar.copy(sc_sb[:], sch[:])
                                    nc.vector.tensor_tensor(
                                        pt[:], sc_sb[:], A_sb[:, i, :],
                                        mybir.AluOpType.mult,
                                    )
                                else:
                                    nc.vector.tensor_tensor(
                                        pt[:], sch[:], A_sb[:, i, :],
                                        mybir.AluOpType.mult,
                                    )
                                pts.append(pt)
                        elif DVE_MERGE:
                            # heads 0-2 share one 3-bank tile (single DVE
                            # consumer); head 3 gets its own bank for the
                            # ScalarE->GPSIMD path
                            scd = psU.tile([P, 3, QB], f32, tag="scd", bufs=1,
                                           name="scd")
                            scg = psU.tile([P, QB], f32, tag="scg",
                                           bufs=SCG_BUFS, name="scg")
                            nc.tensor.matmul(
                                scd[:, 0, :], khT_sb[0:DK, 0, ksl], qhT_cur[0:DK, 0, :],
                                start=True, stop=True, tile_position=(0, 0),
                                skip_group_check=True,
                            )
                            nc.tensor.matmul(
                                scd[:, 1, :], khT_sb[DK:P, 0, ksl], qhT_cur[DK:P, 0, :],
                                start=True, stop=True, tile_position=(DK, 0),
                                skip_group_check=True,
                            )
                            nc.tensor.matmul(
                                scd[:, 2, :], khT_sb[0:DK, 1, ksl], qhT_cur[0:DK, 1, :],
                                start=True, stop=True, tile_position=(0, 0),
                                skip_group_check=True,
                            )
                            nc.tensor.matmul(
                                scg[:], khT_sb[DK:P, 1, ksl], qhT_cur[DK:P, 1, :],
                                start=True, stop=True, tile_position=(DK, 0),
                                skip_group_check=True,
                            )
                            pt3 = sp.tile([P, 3, QB], f16, tag="pt3", bufs=3,
                                          name="pt3")
                            ptg = sp.tile([P, QB], f16, tag="ptg", bufs=3,
                                          name="ptg")
                            nc.vector.tensor_tensor(
                                pt3[:], scd[:], _bcast_mid(A_sb[:, i, :], 3),
                                mybir.AluOpType.mult,
                            )
                            sc_sb = sp.tile([P, QB], f32, tag="scb", bufs=6,
                                            name="sc_sb")
                            nc.scalar.copy(sc_sb[:], scg[:])
                            nc.gpsimd.tensor_tensor(
                                ptg[:], sc_sb[:], A_sb[:, i, :],
                                mybir.AluOpType.mult,
                            )
                            pts = [pt3[:, 0, :], pt3[:, 1, :], pt3[:, 2, :], ptg[:]]
                        else:
                            # scores: both heads of a pair run concurrently in
                            # the upper/lower PE row-quadrants
                            scs = []
                            for j in range(NPAIR):
                                sc_e = psU.tile([P, QB], f32, tag="sc", bufs=SC_BUFS, name="sc_e")
                                nc.tensor.matmul(
                                    sc_e[:], khT_sb[0:DK, j, ksl], qhT_cur[0:DK, j, :],
                                    start=True, stop=True, tile_position=(0, 0),
                                )
                                sc_o = psU.tile([P, QB], f32, tag="sc", bufs=SC_BUFS, name="sc_o")
                                nc.tensor.matmul(
                                    sc_o[:], khT_sb[DK:P, j, ksl], qhT_cur[DK:P, j, :],
                                    start=True, stop=True, tile_position=(DK, 0),
                                )
                                scs += [sc_e, sc_o]
                            # mask multiply, spread over DVE / (ScalarE+GPSIMD):
                            # heads 0,1 on DVE; heads 2,3 alternate by key-block
                            if PV_POS and len(pend_pts) >= PV_DEPTH:
                                emit_xt(*pend_pts.pop(0))
                            pts = []
                            for h in range(HPC):
                                pt = sp.tile([P, QB], f16, tag="pt", bufs=PT_BUFS, name="pt")
                                if MASK_ROUTE:
                                    r = MASK_ROUTE[(kb * HPC + h) % len(MASK_ROUTE)] \
                                        if len(MASK_ROUTE) > HPC else MASK_ROUTE[h]
                                    if QB0_PLAIN and qb == 0 and r == "a":
                                        r = "d"
                                    use_gp = r == "g"
                                    use_assist = r == "a"
                                else:
                                    gmod = GP_MOD1 if (GP_MOD1 and qb > 0) else GP_MOD
                                    use_gp = h == 3 or (h == 2 and kb % gmod == 0)
                                    use_assist = ACT_ASSIST and h == 2 and not use_gp
                                if use_assist:
                                    # ScalarE casts scores to f16 in SBUF, DVE
                                    # multiplies in 4x all-SBUF mode
                                    sc_sb = sp.tile([P, QB], f16, tag="scbh", bufs=4,
                                                    name="sc_sbh")
                                    nc.scalar.copy(sc_sb[:], scs[h][:])
                                    nc.vector.tensor_tensor(
                                        pt[:], sc_sb[:], A_sb[:, i, :],
                                        mybir.AluOpType.mult,
                                    )
                                elif use_gp and GP_PSUM:
                                    nc.gpsimd.tensor_tensor(
                                        pt[:], scs[h][:], A_sb[:, i, :],
                                        mybir.AluOpType.mult,
                                    )
                                elif use_gp:
                                    sc_sb = sp.tile([P, QB], f32, tag="scb", bufs=SCB_BUFS,
                                                    name="sc_sb")
                                    if BOUNCE_HALF:
                                        for hf in range(2):
                                            fsl = slice(hf * (QB // 2), (hf + 1) * (QB // 2))
                                            nc.scalar.copy(sc_sb[:, fsl], scs[h][:, fsl])
                                            nc.gpsimd.tensor_tensor(
                                                pt[:, fsl], sc_sb[:, fsl], A_sb[:, i, fsl],
                                                mybir.AluOpType.mult,
                                            )
                                    else:
                                        nc.scalar.copy(sc_sb[:], scs[h][:])
                                        nc.gpsimd.tensor_tensor(
                                            pt[:], sc_sb[:], A_sb[:, i, :],
                                            mybir.AluOpType.mult,
                                        )
                                else:
                                    nc.vector.tensor_tensor(
                                        pt[:], scs[h][:], A_sb[:, i, :],
                                        mybir.AluOpType.mult,
                                    )
                                pts.append(pt)
                        # software pipeline: emit an older key block's p@v
                        # matmuls now, so the PE never waits mid-iteration for
                        # this kb's mask mults
                        pend_pts.append((pts, kb))
                        if not PV_POS and len(pend_pts) >= PV_DEPTH:
                            emit_xt(*pend_pts.pop(0))
                        for _ in range(FILL_RATE):
                            if fillers:
                                fillers.pop(0)()


                while fillers:
                    fillers.pop(0)()
                while pend_pts:
                    emit_xt(*pend_pts.pop(0))  # drain the remaining key blocks
                pend_xts = (xts, qb)
                qhT_cur, qhT_next = qhT_next, None

            # drain the last query block's output projection through the
            # score banks (idle by now) for deeper tail pipelining
            xts_p, qb_p = pend_xts
            ep_tag, ep_bufs = ("u", U_BUFS) if (DVE_MERGE or SC_PAIR) \
                else ("sc", SC_BUFS)
            for ssub in range(QB // P):
                o_proj_chain(xts_p, qb_p, ssub, ptag=ep_tag, pbufs=ep_bufs,
                             epi=True)

    nc.compile()
    return nc


def _numpy_fallback(q, k, v, A, Wq, bq, Wk, bk, Wv, bv, Wo, bo):
    def proj(x, W, b):
        y = x @ W.T + b
        return y.reshape(B, S, H, DK).transpose(0, 2, 1, 3)

    qh, kh, vh = proj(q, Wq, bq), proj(k, Wk, bk), proj(v, Wv, bv)
    scores = np.einsum("bhqd,bhkd->bhqk", qh, kh) * np.float32(SCALE)
    p = scores * A.T
    x = np.einsum("bhqk,bhkd->bhqd", p, vh)
    x = x.transpose(0, 2, 1, 3).reshape(B, S, D)
    return (x @ Wo.T + bo).astype(np.float32)


def kernel(**inputs):
    q = np.asarray(inputs["q"], dtype=np.float32)
    k = np.asarray(inputs["k"], dtype=np.float32)
    v = np.asarray(inputs["v"], dtype=np.float32)
    A = np.asarray(inputs["A"], dtype=np.float32)
    Wq = np.asarray(inputs["Wq"], dtype=np.float32)
    Wk = np.asarray(inputs["Wk"], dtype=np.float32)
    Wv = np.asarray(inputs["Wv"], dtype=np.float32)
    Wo = np.asarray(inputs["Wo"], dtype=np.float32)
    bq, bk, bv, bo = (np.asarray(inputs[n], dtype=np.float32) for n in ("bq", "bk", "bv", "bo"))

    # The device kernel folds zero biases away (spec fills them with zeros);
    # fall back to a host reference in the (unused) nonzero-bias case.
    if any(np.any(b) for b in (bq, bk, bv)):
        return _numpy_fallback(q, k, v, A, Wq, bq, Wk, bk, Wv, bv, Wo, bo)

    global _CACHED
    if _CACHED is None:
        _CACHED = _build()
    nc = _CACHED

    Asc = np.ascontiguousarray((A * np.float32(SCALE)).astype(np.float16))
    in_maps = []
    for c in range(NCORES):
        b, g = divmod(c, GROUPS)
        hsl = slice(g * HD, (g + 1) * HD)
        in_maps.append({
            "qT": np.ascontiguousarray(q[b].T.astype(np.float16)),
            "kT": np.ascontiguousarray(k[b].T.astype(np.float16)),
            "vT": np.ascontiguousarray(v[b].T.astype(np.float16)),
            "Asc": Asc,
            "wq": np.ascontiguousarray(Wq[hsl].T.astype(np.float16)),
            "wk": np.ascontiguousarray(Wk[hsl].T.astype(np.float16)),
            "wv": np.ascontiguousarray(Wv[hsl].T.astype(np.float16)),
            "wo": np.ascontiguousarray(
                Wo[:, hsl].T.astype(np.float16) if WO_F16 else Wo[:, hsl].T),
        })

    res = bass_utils.run_bass_kernel_spmd(
        nc, in_maps, core_ids=list(range(NCORES)), trace=TRACE
    )
    global LAST_RESULTS
    LAST_RESULTS = res

    out = np.zeros((B, S, D), dtype=np.float32)
    for c in range(NCORES):
        out[c // GROUPS] += res.results[c]["out"].astype(np.float32)
    out += bo
    return out


if __name__ == "__main__":
    rng = np.random.default_rng(0)
    ins = {
        "q": rng.standard_normal((B, S, D), dtype=np.float32),
        "k": rng.standard_normal((B, S, D), dtype=np.float32),
        "v": rng.standard_normal((B, S, D), dtype=np.float32),
        "A": rng.random((S, S), dtype=np.float32),
        "Wq": rng.standard_normal((D, D), dtype=np.float32) / 32,
        "bq": np.zeros(D, np.float32),
        "Wk": rng.standard_normal((D, D), dtype=np.float32) / 32,
        "bk": np.zeros(D, np.float32),
        "Wv": rng.standard_normal((D, D), dtype=np.float32) / 32,
        "bv": np.zeros(D, np.float32),
        "Wo": rng.standard_normal((D, D), dtype=np.float32) / 32,
        "bo": np.zeros(D, np.float32),
    }
    got = kernel(**ins)
    ref = _numpy_fallback(**ins)
    err = np.abs(got - ref).max() / np.abs(ref).max()
    print("self-check relmax:", err)



# revision 14
# speedup vs baseline: 1.0387x; 1.0387x over previous
"""Trainium2 Bass kernel for nn_AttentionBlock (sparse_attention, no-softmax).

Computation (per batch b):
    qh = (q @ Wq^T) split into 16 heads of dk=64     [S, D] -> [H, S, DK]
    kh, vh likewise
    scores = (qh @ kh^T) / sqrt(DK)                  [H, S, S]
    p      = scores * A^T                            (elementwise structural mask)
    x      = p @ vh                                  [H, S, DK] -> [S, D]
    out    = x @ Wo^T + bo                           [S, D]

Sharding over 8 NeuronCores: data-parallel over batch (B=2) x tensor-parallel
over heads (16 heads -> 4 per core). Each core projects q/k/v for its 4 heads
(column-parallel), runs masked attention for them, and applies its 256-column
slice of the output projection (row-parallel), producing a full-shape partial
output. Host sums the 4 partials per batch.

Implementation notes:
- Activations are shipped pre-transposed ([D, S]) so every matmul contraction
  dim lands on SBUF partitions with no on-device transposes; 1/sqrt(DK) is
  folded into the mask A on the host.
- The whole data path runs in fp16 with fp32 PSUM accumulation (all operands
  here are O(1)-O(100), well inside fp16 range; measured end-to-end error is
  a few 1e-4). fp16 is the same PE stream rate as bf16/f32r but, being
  2-byte, additionally halves DMA/SBUF traffic and legalizes PE quadrant
  packing (tile_position), which f32/f32r reject.
- Heads are stored as pairs on the partition axis (head 2j on partitions
  0:63, head 2j+1 on 64:127). The K=64 score matmuls of a pair run
  concurrently in the upper/lower PE row-quadrants (tile_position (0,0) /
  (64,0)); the M=64 p@v matmuls of a pair run concurrently in left/right
  col-quadrants into one PSUM bank (tile_position (0,0) / (0,64)).
- The mask multiply is the throughput-critical elementwise stage; it is
  spread over three engines per the MASK_ROUTE pattern: DVE straight out of
  PSUM (most tiles), a ScalarE PSUM->SBUF bounce feeding GPSIMD (head 3 +
  head 2 every 8th key block), and occasionally a ScalarE f16 bounce feeding
  a DVE 2x all-SBUF multiply. GPSIMD cannot read PSUM (neuronxcc rejects
  it), so its tiles must bounce through ScalarE.
- Projection work for the next/previous query block is interleaved into the
  attention loop so no engine drains the pipeline at block boundaries.
  The prologue orders DMAs by true need time (k weights/activations, then
  q, then v, then the mask tiles); slice-0 v-chains drain as group-0
  fillers so scores start ~13us in. Partial outputs and the o-projection
  weights are fp16 (halves outbound DMA; host accumulates in fp32), and
  the epilogue alternates its PSUM->SBUF drains between ScalarE and DVE.
"""

import numpy as np

import concourse.mybir as mybir
import concourse.tile as tile
from concourse import bacc, bass_utils
from concourse.bass import AP


def _bcast_mid(ap2, n):
    """[128, F] AP -> [128, (0-stride n), F]: broadcast over an inserted
    middle dim so one tensor_tensor applies the same mask row-block to n
    head slots."""
    lay = [list(d) for d in ap2.ap]
    assert len(lay) == 2
    return AP(ap2.tensor, ap2.offset, [lay[0], [0, n], lay[1]])

B, S, D, H = 2, 2048, 1024, 16
NCORES = 8
GROUPS = NCORES // B          # 4 head-groups
HPC = H // GROUPS             # 4 heads per core
DK = D // H                   # 64
HD = HPC * DK                 # 256 head-dim columns per core
NPAIR = HPC // 2              # 2 head pairs per core
SCALE = 1.0 / np.sqrt(DK)

P = 128                       # SBUF partitions
QB = 512                      # query block
NQB = S // QB                 # 4
KBLK = 128                    # key block
NKB = S // KBLK               # 16
NKT = D // P                  # 8 contraction chunks for projections
AGRP = 4                      # key-blocks per A-tile DMA / interleave group
NGRP = NKB // AGRP            # 4 groups

f32 = mybir.dt.float32
f16 = mybir.dt.float16

import os
KV_SPLIT = int(os.environ.get("KV_SPLIT", "1"))       # split k/q chains in half
MERGE_KQ = int(os.environ.get("MERGE_KQ", "1"))       # merged khT/qhT drains
MERGE_XTS = int(os.environ.get("MERGE_XTS", "0"))     # merged xts drain
OPROJ_SPLIT = int(os.environ.get("OPROJ_SPLIT", "1")) # o-proj per-et thunks
A_PREF = int(os.environ.get("A_PREF", "1"))           # A prefetch depth (1|2)
GP_MOD = int(os.environ.get("GP_MOD", "8"))           # head2 on GP when kb%GP_MOD==0
OUT_F16 = int(os.environ.get("OUT_F16", "1"))         # fp16 partial outputs
DVE_MERGE = int(os.environ.get("DVE_MERGE", "0"))     # heads 0-2 in one DVE mask op
SCG_BUFS = int(os.environ.get("SCG_BUFS", "1"))       # psum bufs for the GP head's scores
U_BUFS = int(os.environ.get("U_BUFS", "2"))           # psum bufs for proj chains
SC_BUFS = int(os.environ.get("SC_BUFS", "4"))         # psum bufs for score tiles
BOUNCE_HALF = int(os.environ.get("BOUNCE_HALF", "0")) # split ScalarE->GPSIMD path in halves
GP_PSUM = int(os.environ.get("GP_PSUM", "0"))         # GPSIMD reads scores straight from PSUM
OPROJ_LATE = int(os.environ.get("OPROJ_LATE", "0"))   # last qb: prev o-proj in late groups
ACT_ASSIST = int(os.environ.get("ACT_ASSIST", "0"))   # h2 via Act f16-bounce + DVE 4x mult
MASK_ROUTE = os.environ.get("MASK_ROUTE", "dddgdddgdgdddgdadgggdddgddggddagddagdaggddgdddggdddagdagddagdddg")         # per-head route: d=DVE, a=Act+DVE2x, g=GPSIMD
WO_F16 = int(os.environ.get("WO_F16", "1"))           # f16 output projection weights + xts
WO_DEFER = int(os.environ.get("WO_DEFER", "1"))       # load wo during qb0 (not prologue)
EPI_DVE = int(os.environ.get("EPI_DVE", "1"))         # epilogue osb copies alternate DVE/Act
GP_MOD1 = int(os.environ.get("GP_MOD1", "0"))         # GP_MOD override for qb>=1 (0=same)
EPI_DMA_SPLIT = int(os.environ.get("EPI_DMA_SPLIT", "0"))  # epilogue DMA per 512-col chunk
PRO_QFIRST = int(os.environ.get("PRO_QFIRST", "1"))   # prologue: q proj before v proj
V_FILLER = int(os.environ.get("V_FILLER", "0"))       # slice-0 v chains as group-0 fillers
A_EARLY = int(os.environ.get("A_EARLY", "0"))
A_AFTER_KV = int(os.environ.get("A_AFTER_KV", "1"))   # group A-prefetch after kv DMAs         # A0/A1 before the v stream
PT_BUFS = int(os.environ.get("PT_BUFS", "32"))        # sbuf bufs for mask outputs
SCB_BUFS = int(os.environ.get("SCB_BUFS", "12"))       # sbuf bufs for GP bounce tiles
SC_PAIR = int(os.environ.get("SC_PAIR", "0"))         # heads 0+1 share a 2-bank score tile
PRO_SPLIT2 = int(os.environ.get("PRO_SPLIT2", "0"))   # finer first wk/kT0 DMA chunks
QB0_PLAIN = int(os.environ.get("QB0_PLAIN", "0"))     # no Act-assist routing in qb0
PV_DEPTH = int(os.environ.get("PV_DEPTH", "7"))       # p@v software-pipeline depth (key blocks)
QT_SPLIT = int(os.environ.get("QT_SPLIT", "1"))       # split qT loads in kt halves
FILL_RATE = int(os.environ.get("FILL_RATE", "2"))     # filler thunks drained per key block
PV_POS = int(os.environ.get("PV_POS", "0"))           # emit pipelined p@v before the masks
WARMUP = int(os.environ.get("WARMUP", "0"))           # dummy matmul at t~0 to start PE p-state ramp
PRO_V2 = int(os.environ.get("PRO_V2", "0"))           # interleaved k/q prologue DMA + chain order
XTS_DVE = int(os.environ.get("XTS_DVE", "0"))         # xts drains: 0=Act,1=DVE,2=split
QDRAIN_DVE = int(os.environ.get("QDRAIN_DVE", "0"))   # qhT drains on DVE
OSB_DVE = int(os.environ.get("OSB_DVE", "0"))         # mid-kernel osb copies: DVE every Nth et
OPROJ_SPREAD = int(os.environ.get("OPROJ_SPREAD", "1"))  # 1 o-proj chain per group (not 2+2)
                                                      # and one merged DVE mask multiply
                                                      # e.g. "daag"; empty = legacy GP_MOD/ACT_ASSIST

_CACHED = None  # built module, reused across kernel() calls
TRACE = False         # set True (e.g. from test.py) to profile the NEFF
LAST_RESULTS = None   # BassKernelResults of the most recent run


def _build():
    nc = bacc.Bacc("TRN2", target_bir_lowering=False)

    qT = nc.dram_tensor("qT", [D, S], f16, kind="ExternalInput")
    kT = nc.dram_tensor("kT", [D, S], f16, kind="ExternalInput")
    vT = nc.dram_tensor("vT", [D, S], f16, kind="ExternalInput")
    Asc = nc.dram_tensor("Asc", [S, S], f16, kind="ExternalInput")
    wq = nc.dram_tensor("wq", [D, HD], f16, kind="ExternalInput")
    wk = nc.dram_tensor("wk", [D, HD], f16, kind="ExternalInput")
    wv = nc.dram_tensor("wv", [D, HD], f16, kind="ExternalInput")
    wo = nc.dram_tensor("wo", [HD, D],
                        f16 if WO_F16 else mybir.dt.float32r, kind="ExternalInput")
    # fp16 partial outputs: halves the outbound DMA; the host accumulates the
    # four per-batch partials in fp32 (adds ~1e-4 relative error)
    out = nc.dram_tensor("out", [S, D], f16 if OUT_F16 else f32, kind="ExternalOutput")

    qT_r = qT.rearrange("(kt p) s -> p kt s", p=P)
    kT_r = kT.rearrange("(kt p) s -> p kt s", p=P)
    vT_r = vT.rearrange("(kt p) s -> p kt s", p=P)
    wq_r = wq.rearrange("(kt p) c -> p kt c", p=P)
    wk_r = wk.rearrange("(kt p) c -> p kt c", p=P)
    wv_r = wv.rearrange("(kt p) c -> p kt c", p=P)
    wo_r = wo.rearrange("(ck p) e -> p ck e", p=P)
    A_r = Asc.rearrange("(kb p) q -> p kb q", p=P)

    with tile.TileContext(nc) as tc:
        with (
            tc.tile_pool(name="persist", bufs=1) as pp,
            tc.tile_pool(name="stream", bufs=2) as sp,
            tc.tile_pool(name="psU", bufs=2, space="PSUM") as psU,   # proj [128,512] x2 + sc x4
            tc.tile_pool(name="psX", bufs=1, space="PSUM") as psX,   # xT accumulators
        ):
            if WARMUP:
                # touch the PE immediately: pe_busy_start is sticky, so one
                # tiny matmul at t~0 means the p-state ramp competes with the
                # prologue DMAs instead of the first real chains
                wu_sb = pp.tile([P, 16], f16, tag="warm")
                nc.vector.memset(wu_sb[:], 0.0)
                wu_ps = psU.tile([P, 16], f32, tag="u", bufs=U_BUFS, name="warm")
                nc.tensor.matmul(wu_ps[0:16, :], wu_sb[:], wu_sb[:],
                                 start=True, stop=True)

            wk_sb = pp.tile([P, NKT, HD], f16, tag="wk")
            wv_sb = pp.tile([P, NKT, HD], f16, tag="wv")
            wq_sb = pp.tile([P, NKT, HD], f16, tag="wq")
            wo_sb = pp.tile([P, HD // P, D],
                            f16 if WO_F16 else mybir.dt.float32r, tag="wo")

            # head-PAIR layout: pair j holds head 2j on partitions 0:64 and
            # head 2j+1 on 64:128 — the layout quadrant packing requires
            khT_sb = pp.tile([P, NPAIR, S], f16, tag="khT")
            vh_sb = pp.tile([P, NKB, HD], f16, tag="vh")     # [ks%128, kb, c]

            def kv_proj_chains(st, kT_pre=None, vT_pre=None):
                """Issue k/v DMAs for s-slice st now; return per-chain thunks
                so the projection matmuls can be sprinkled between attention
                iterations instead of lumped at group boundaries."""
                sl = slice(st * QB, (st + 1) * QB)
                kT_sb = kT_pre
                if kT_sb is None:
                    kT_sb = sp.tile([P, NKT, QB], f16, tag="xin", bufs=4, name="kT_sb")
                    nc.sync.dma_start(kT_sb[:], kT_r[:, :, sl])
                vT_sb = vT_pre
                if vT_sb is None:
                    vT_sb = sp.tile([P, NKT, QB], f16, tag="xin", bufs=4, name="vT_sb")
                    nc.sync.dma_start(vT_sb[:], vT_r[:, :, sl])

                pk_tiles = {}

                def kchain_a(ct, kT_sb=kT_sb):
                    pk = psU.tile([P, QB], f32, tag="u", bufs=U_BUFS, name="pk")
                    pk_tiles[ct] = pk
                    hi = NKT // 2 if KV_SPLIT else NKT
                    for kt in range(hi):
                        nc.tensor.matmul(
                            pk[:], wk_sb[:, kt, ct * P:(ct + 1) * P], kT_sb[:, kt, :],
                            start=(kt == 0), stop=(kt == NKT - 1),
                        )
                    if not KV_SPLIT:
                        kdrain(ct)

                def kdrain(ct):
                    pk = pk_tiles.pop(ct)
                    if MERGE_KQ:
                        nc.scalar.copy(khT_sb[:, ct, sl], pk[:])
                    else:
                        nc.scalar.copy(khT_sb[0:DK, ct, sl], pk[0:DK, :])
                        nc.scalar.copy(khT_sb[DK:P, ct, sl], pk[DK:P, :])

                def kchain_b(ct, kT_sb=kT_sb):
                    pk = pk_tiles[ct]
                    for kt in range(NKT // 2, NKT):
                        nc.tensor.matmul(
                            pk[:], wk_sb[:, kt, ct * P:(ct + 1) * P], kT_sb[:, kt, :],
                            start=False, stop=(kt == NKT - 1),
                        )
                    kdrain(ct)

                def vchain(ssub, vT_sb=vT_sb):
                    kb = st * (QB // P) + ssub
                    pv = psU.tile([P, HD], f32, tag="u", bufs=U_BUFS, name="pv")
                    for kt in range(NKT):
                        nc.tensor.matmul(
                            pv[:], vT_sb[:, kt, ssub * P:(ssub + 1) * P], wv_sb[:, kt, :],
                            start=(kt == 0), stop=(kt == NKT - 1),
                        )
                    nc.scalar.copy(vh_sb[:, kb, :], pv[:])

                thunks = []
                for ct in range(NPAIR):
                    thunks.append(lambda ct=ct: kchain_a(ct))
                    if KV_SPLIT:
                        thunks.append(lambda ct=ct: kchain_b(ct))
                thunks += [lambda s=s: vchain(s) for s in range(QB // P)]
                return thunks

            def kv_proj(st, kT_pre=None, vT_pre=None):
                for thunk in kv_proj_chains(st, kT_pre, vT_pre):
                    thunk()

            def q_proj_chains(qb, qT_pre=None):
                """Issue the q DMA now; return (qhT tile, per-pair chain thunks)."""
                qsl = slice(qb * QB, (qb + 1) * QB)
                qT_sb = qT_pre
                if qT_sb is None:
                    qT_sb = sp.tile([P, NKT, QB], f16, tag="xin", bufs=4, name="qT_sb")
                    if QT_SPLIT:
                        nc.sync.dma_start(qT_sb[:, 0:NKT // 2, :], qT_r[:, 0:NKT // 2, qsl])
                        nc.sync.dma_start(qT_sb[:, NKT // 2:, :], qT_r[:, NKT // 2:, qsl])
                    else:
                        nc.sync.dma_start(qT_sb[:], qT_r[:, :, qsl])
                qhT_sb = sp.tile([P, NPAIR, QB], f16, tag="qh", bufs=3, name="qhT_sb")

                pq_tiles = {}

                def chain_a(ct):
                    pq = psU.tile([P, QB], f32, tag="u", bufs=U_BUFS, name="pq")
                    pq_tiles[ct] = pq
                    hi = NKT // 2 if KV_SPLIT else NKT
                    for kt in range(hi):
                        nc.tensor.matmul(
                            pq[:], wq_sb[:, kt, ct * P:(ct + 1) * P], qT_sb[:, kt, :],
                            start=(kt == 0), stop=(kt == NKT - 1),
                        )
                    if not KV_SPLIT:
                        qdrain(ct)

                def qdrain(ct):
                    pq = pq_tiles.pop(ct)
                    if QDRAIN_DVE:
                        nc.vector.tensor_copy(qhT_sb[:, ct, :], pq[:])
                    elif MERGE_KQ:
                        nc.scalar.copy(qhT_sb[:, ct, :], pq[:])
                    else:
                        nc.scalar.copy(qhT_sb[0:DK, ct, :], pq[0:DK, :])
                        nc.scalar.copy(qhT_sb[DK:P, ct, :], pq[DK:P, :])

                def chain_b(ct):
                    pq = pq_tiles[ct]
                    for kt in range(NKT // 2, NKT):
                        nc.tensor.matmul(
                            pq[:], wq_sb[:, kt, ct * P:(ct + 1) * P], qT_sb[:, kt, :],
                            start=False, stop=(kt == NKT - 1),
                        )
                    qdrain(ct)

                thunks = []
                for ct in range(NPAIR):
                    thunks.append(lambda ct=ct: chain_a(ct))
                    if KV_SPLIT:
                        thunks.append(lambda ct=ct: chain_b(ct))
                return qhT_sb, thunks

            def q_proj(qb):
                qhT_sb, chains = q_proj_chains(qb)
                for thunk in chains:
                    thunk()
                return qhT_sb

            def o_proj_et(xts, qb, ssub, et, osb, ptag="u", pbufs=None,
                          dve_copy=False, epi_split=False):
                """One 512-col chunk of the output projection for query block
                qb, row-slice ssub; et==last also issues the out DMA."""
                if pbufs is None and ptag == "u":
                    pbufs = U_BUFS
                po = psU.tile([P, QB], f32, tag=ptag, bufs=pbufs, name="po")
                for ck in range(HD // P):
                    nc.tensor.matmul(
                        po[:],
                        xts[:, ck, ssub * P:(ssub + 1) * P],
                        wo_sb[:, ck, et * QB:(et + 1) * QB],
                        start=(ck == 0), stop=(ck == HD // P - 1),
                    )
                if dve_copy or (OSB_DVE and et % OSB_DVE == 0):
                    nc.vector.tensor_copy(osb[:, et * QB:(et + 1) * QB], po[:])
                else:
                    nc.scalar.copy(osb[:, et * QB:(et + 1) * QB], po[:])
                rsl = slice(qb * QB + ssub * P, qb * QB + (ssub + 1) * P)
                if epi_split:
                    nc.sync.dma_start(
                        out[rsl, et * QB:(et + 1) * QB],
                        osb[:, et * QB:(et + 1) * QB],
                    )
                elif et == D // QB - 1:
                    nc.sync.dma_start(out[rsl, :], osb[:])

            def o_proj_chain_thunks(xts, qb, ssub, ptag="u", pbufs=None,
                                    epi=False):
                osb = sp.tile([P, D], f16 if OUT_F16 else f32, tag="osb", bufs=4, name="osb")
                ets = [
                    lambda et=et: o_proj_et(xts, qb, ssub, et, osb, ptag, pbufs,
                                            dve_copy=epi and EPI_DVE and et % 2 == 0,
                                            epi_split=epi and EPI_DMA_SPLIT)
                    for et in range(D // QB)
                ]
                if OPROJ_SPLIT:
                    return ets
                def whole():
                    for t in ets:
                        t()
                return [whole]

            def o_proj_chain(xts, qb, ssub, ptag="u", pbufs=None, epi=False):
                for thunk in o_proj_chain_thunks(xts, qb, ssub, ptag, pbufs, epi):
                    thunk()

            # ---- pipeline ---------------------------------------------------------
            # prologue DMAs in dependency-first order so the PE starts ASAP
            kT0 = sp.tile([P, NKT, QB], f16, tag="xin", bufs=4, name="kT_sb")
            if PRO_V2:
                # interleave the k and q projection DMA streams so both paths'
                # chain_a/chain_b pairs consume data as it lands; the first
                # score matmul is gated by the LAST proj drain, so balancing
                # the two DMA paths beats finishing one path first
                H2 = NKT // 2
                qT0 = sp.tile([P, NKT, QB], f16, tag="xin", bufs=4, name="qT_sb")
                vT0 = sp.tile([P, NKT, QB], f16, tag="xin", bufs=4, name="vT_sb")
                nc.sync.dma_start(wk_sb[:, 0:H2, :], wk_r[:, 0:H2, :])
                nc.sync.dma_start(kT0[:, 0:H2, :], kT_r[:, 0:H2, 0:QB])
                nc.sync.dma_start(wq_sb[:, 0:H2, :], wq_r[:, 0:H2, :])
                nc.sync.dma_start(qT0[:, 0:H2, :], qT_r[:, 0:H2, 0:QB])
                nc.sync.dma_start(wk_sb[:, H2:, :], wk_r[:, H2:, :])
                nc.sync.dma_start(kT0[:, H2:, :], kT_r[:, H2:, 0:QB])
                nc.sync.dma_start(wq_sb[:, H2:, :], wq_r[:, H2:, :])
                nc.sync.dma_start(qT0[:, H2:, :], qT_r[:, H2:, 0:QB])
                fetch_A_early = True
            else:
                fetch_A_early = False
            if not PRO_V2:
                if PRO_SPLIT2:
                    nc.sync.dma_start(wk_sb[:, 0:2, :], wk_r[:, 0:2, :])
                    nc.sync.dma_start(kT0[:, 0:2, :], kT_r[:, 0:2, 0:QB])
                    nc.sync.dma_start(wk_sb[:, 2:NKT // 2, :], wk_r[:, 2:NKT // 2, :])
                    nc.sync.dma_start(kT0[:, 2:NKT // 2, :], kT_r[:, 2:NKT // 2, 0:QB])
                else:
                    nc.sync.dma_start(wk_sb[:, 0:NKT // 2, :], wk_r[:, 0:NKT // 2, :])
                    nc.sync.dma_start(kT0[:, 0:NKT // 2, :], kT_r[:, 0:NKT // 2, 0:QB])
                nc.sync.dma_start(wk_sb[:, NKT // 2:, :], wk_r[:, NKT // 2:, :])
                nc.sync.dma_start(kT0[:, NKT // 2:, :], kT_r[:, NKT // 2:, 0:QB])
                if not PRO_QFIRST:
                    nc.sync.dma_start(wv_sb[:], wv_r[:])
                vT0 = sp.tile([P, NKT, QB], f16, tag="xin", bufs=4, name="vT_sb")
                if not PRO_QFIRST:
                    nc.sync.dma_start(vT0[:], vT_r[:, :, 0:QB])
                nc.sync.dma_start(wq_sb[:], wq_r[:])

            # A tiles are fetched two groups ahead; group sequence is linear
            # over (qb, kbg)
            gseq = [(qb, kbg) for qb in range(NQB) for kbg in range(NGRP)]
            A_tiles = {}

            def fetch_A(gidx):
                if gidx >= len(gseq):
                    return
                aqb, akbg = gseq[gidx]
                A_sb = sp.tile([P, AGRP, QB], f16, tag="A", bufs=4, name="A_sb")
                nc.sync.dma_start(
                    A_sb[:],
                    A_r[:, akbg * AGRP:(akbg + 1) * AGRP, aqb * QB:(aqb + 1) * QB],
                )
                A_tiles[gidx] = A_sb

            pend_fillers = []
            if PRO_V2:
                # PE order: kchain_a pair, qchain_a pair (consume first DMA
                # halves), then kchain_b / qchain_b (+drains) on the second
                # halves; A and the v stream follow the proj DMAs
                qhT_cur, qchains0 = q_proj_chains(0, qT_pre=qT0)
                fetch_A(0)
                if A_PREF >= 2:
                    fetch_A(1)
                nc.sync.dma_start(wv_sb[:], wv_r[:])
                nc.sync.dma_start(vT0[:], vT_r[:, :, 0:QB])
                thunks0 = kv_proj_chains(0, kT_pre=kT0, vT_pre=vT0)
                assert KV_SPLIT, "PRO_V2 requires split chains"
                # interleave: ka0 ka1 | qa0 qa1 | kb0 kb1 | qb0 qb1
                for t in (thunks0[0], thunks0[2]):   # kchain_a ct0, ct1
                    t()
                for t in (qchains0[0], qchains0[2]):  # qchain_a ct0, ct1
                    t()
                for t in (thunks0[1], thunks0[3]):   # kchain_b ct0, ct1
                    t()
                for t in (qchains0[1], qchains0[3]):  # qchain_b ct0, ct1
                    t()
                if V_FILLER:
                    pend_fillers = thunks0[2 * NPAIR:]
                else:
                    for t in thunks0[2 * NPAIR:]:
                        t()
            elif PRO_QFIRST:
                # q projection (and its DMA) before the v stream: scores can
                # start ~6us earlier; v data is not needed until the first p@v
                qhT_cur, qchains0 = q_proj_chains(0)
                if A_EARLY:
                    fetch_A(0)
                    if A_PREF >= 2:
                        fetch_A(1)
                nc.sync.dma_start(wv_sb[:], wv_r[:])
                nc.sync.dma_start(vT0[:], vT_r[:, :, 0:QB])
                if not A_EARLY:
                    fetch_A(0)
                    if A_PREF >= 2:
                        fetch_A(1)
                thunks0 = kv_proj_chains(0, kT_pre=kT0, vT_pre=vT0)
                for t in thunks0[:2 * NPAIR]:   # k chains
                    t()
                for t in qchains0:              # q chains
                    t()
                if V_FILLER:
                    pend_fillers = thunks0[2 * NPAIR:]
                else:
                    for t in thunks0[2 * NPAIR:]:   # v chains
                        t()
            else:
                fetch_A(0)
                kv_proj(0, kT_pre=kT0, vT_pre=vT0)
                qhT_cur = q_proj(0)
                if A_PREF >= 2:
                    fetch_A(1)
            if not WO_DEFER:
                nc.sync.dma_start(wo_sb[:], wo_r[:])

            pend_xts = None    # (xts tile, qb) awaiting output projection
            qhT_next = None

            for qb in range(NQB):
                qsl = slice(qb * QB, (qb + 1) * QB)
                xt = psX.tile([P, NPAIR, QB], f32, tag="xt", name="xt")  # 2 banks
                xts = sp.tile([P, NPAIR, QB],
                              f16 if WO_F16 else mybir.dt.float32r,
                              tag="xts", bufs=3, name="xts")
                pend_pts = []

                def emit_xt(pts, kb, xt=xt, xts=xts):
                    # p @ v: both heads of a pair run concurrently in the
                    # left/right PE col-quadrants into one PSUM bank.
                    # skip_group_check: the two col-quadrant groups legally
                    # share one PSUM bank (sim-only guard).
                    for j in range(NPAIR):
                        nc.tensor.matmul(
                            xt[0:DK, j, :],
                            vh_sb[:, kb, (2 * j) * DK:(2 * j + 1) * DK],
                            pts[2 * j],
                            start=(kb == 0), stop=(kb == NKB - 1),
                            tile_position=(0, 0), skip_group_check=True,
                        )
                        nc.tensor.matmul(
                            xt[DK:P, j, :],
                            vh_sb[:, kb, (2 * j + 1) * DK:(2 * j + 2) * DK],
                            pts[2 * j + 1],
                            start=(kb == 0), stop=(kb == NKB - 1),
                            tile_position=(0, DK), skip_group_check=True,
                        )
                    if kb == NKB - 1:
                        if MERGE_XTS:
                            nc.scalar.copy(xts[:], xt[:])
                        elif XTS_DVE == 1:
                            nc.vector.tensor_copy(xts[:, 0, :], xt[:, 0, :])
                            nc.vector.tensor_copy(xts[:, 1, :], xt[:, 1, :])
                        elif XTS_DVE == 2:
                            nc.scalar.copy(xts[:, 0, :], xt[:, 0, :])
                            nc.vector.tensor_copy(xts[:, 1, :], xt[:, 1, :])
                        else:
                            nc.scalar.copy(xts[:, 0, :], xt[:, 0, :])
                            nc.scalar.copy(xts[:, 1, :], xt[:, 1, :])

                fillers = list(pend_fillers)
                pend_fillers = []
                for kbg in range(NGRP):
                    gidx = qb * NGRP + kbg
                    if gidx not in A_tiles:
                        fetch_A(gidx)
                    A_sb = A_tiles.pop(gidx)
                    if not A_AFTER_KV:
                        fetch_A(gidx + A_PREF)
                    # queue this group's independent projection work; it is
                    # drained two chains per key-block below, keeping the PE
                    # fed without starving the elementwise engines
                    if qb == 0:
                        if kbg == 2 and WO_DEFER:
                            nc.sync.dma_start(wo_sb[:], wo_r[:])
                        if kbg < NGRP - 1:
                            fillers += kv_proj_chains(kbg + 1)
                        else:
                            qhT_next, qchains = q_proj_chains(1)
                            fillers += qchains
                    else:
                        late = OPROJ_LATE and qb == NQB - 1
                        if OPROJ_SPREAD:
                            og = kbg if not late else kbg - 1
                            lastog = NGRP - 1 if not late else NGRP - 2
                        else:
                            og = kbg - 2 if late else kbg
                            lastog = 1
                        if 0 <= og <= lastog and pend_xts is not None:
                            xts_p, qb_p = pend_xts
                            if OPROJ_SPREAD:
                                fillers += o_proj_chain_thunks(xts_p, qb_p, og)
                            else:
                                fillers += o_proj_chain_thunks(xts_p, qb_p, 2 * og)
                                fillers += o_proj_chain_thunks(xts_p, qb_p, 2 * og + 1)
                            if og == lastog:
                                pend_xts = None
                        if kbg == NGRP - 1 and qb < NQB - 1:
                            qhT_next, qchains = q_proj_chains(qb + 1)
                            fillers += qchains
                    if A_AFTER_KV:
                        fetch_A(gidx + A_PREF)
                    for i in range(AGRP):
                        kb = kbg * AGRP + i
                        ksl = slice(kb * KBLK, (kb + 1) * KBLK)
                        if SC_PAIR:
                            # heads 0,1: quadrant-packed score pair into one
                            # 2-bank tile; both banks finish together, so one
                            # merged DVE multiply adds no latency
                            sc2 = psU.tile([P, 2, QB], f32, tag="sc2", bufs=1,
                                           name="sc2")
                            sch2 = psU.tile([P, QB], f32, tag="sch2", bufs=1,
                                            name="sch2")
                            sch3 = psU.tile([P, QB], f32, tag="sch3", bufs=1,
                                            name="sch3")
                            nc.tensor.matmul(
                                sc2[:, 0, :], khT_sb[0:DK, 0, ksl], qhT_cur[0:DK, 0, :],
                                start=True, stop=True, tile_position=(0, 0),
                                skip_group_check=True,
                            )
                            nc.tensor.matmul(
                                sc2[:, 1, :], khT_sb[DK:P, 0, ksl], qhT_cur[DK:P, 0, :],
                                start=True, stop=True, tile_position=(DK, 0),
                                skip_group_check=True,
                            )
                            nc.tensor.matmul(
                                sch2[:], khT_sb[0:DK, 1, ksl], qhT_cur[0:DK, 1, :],
                                start=True, stop=True, tile_position=(0, 0),
                                skip_group_check=True,
                            )
                            nc.tensor.matmul(
                                sch3[:], khT_sb[DK:P, 1, ksl], qhT_cur[DK:P, 1, :],
                                start=True, stop=True, tile_position=(DK, 0),
                                skip_group_check=True,
                            )
                            pt2 = sp.tile([P, 2, QB], f16, tag="pt2", bufs=3,
                                          name="pt2")
                            nc.vector.tensor_tensor(
                                pt2[:], sc2[:], _bcast_mid(A_sb[:, i, :], 2),
                                mybir.AluOpType.mult,
                            )
                            pts = [pt2[:, 0, :], pt2[:, 1, :]]
                            for h, sch in ((2, sch2), (3, sch3)):
                                pt = sp.tile([P, QB], f16, tag="pt", bufs=PT_BUFS,
                                             name="pt")
                                r = MASK_ROUTE[(kb * HPC + h) % len(MASK_ROUTE)] \
                                    if len(MASK_ROUTE) > HPC else MASK_ROUTE[h]
                                if r == "g":
                                    sc_sb = sp.tile([P, QB], f32, tag="scb",
                                                    bufs=SCB_BUFS, name="sc_sb")
                                    nc.scalar.copy(sc_sb[:], sch[:])
                                    nc.gpsimd.tensor_tensor(
                                        pt[:], sc_sb[:], A_sb[:, i, :],
                                        mybir.AluOpType.mult,
                                    )
                                elif r == "a":
                                    sc_sb = sp.tile([P, QB], f16, tag="scbh", bufs=4,
                                                    name="sc_sbh")
                                    nc.scalar.copy(sc_sb[:], sch[:])
                                    nc.vector.tensor_tensor(
                                        pt[:], sc_sb[:], A_sb[:, i, :],
                                        mybir.AluOpType.mult,
                                    )
                                else:
                                    nc.vector.tensor_tensor(
                                        pt[:], sch[:], A_sb[:, i, :],
                                        mybir.AluOpType.mult,
                                    )
                                pts.append(pt)
                        elif DVE_MERGE:
                            # heads 0-2 share one 3-bank tile (single DVE
                            # consumer); head 3 gets its own bank for the
                            # ScalarE->GPSIMD path
                            scd = psU.tile([P, 3, QB], f32, tag="scd", bufs=1,
                                           name="scd")
                            scg = psU.tile([P, QB], f32, tag="scg",
                                           bufs=SCG_BUFS, name="scg")
                            nc.tensor.matmul(
                                scd[:, 0, :], khT_sb[0:DK, 0, ksl], qhT_cur[0:DK, 0, :],
                                start=True, stop=True, tile_position=(0, 0),
                                skip_group_check=True,
                            )
                            nc.tensor.matmul(
                                scd[:, 1, :], khT_sb[DK:P, 0, ksl], qhT_cur[DK:P, 0, :],
                                start=True, stop=True, tile_position=(DK, 0),
                                skip_group_check=True,
                            )
                            nc.tensor.matmul(
                                scd[:, 2, :], khT_sb[0:DK, 1, ksl], qhT_cur[0:DK, 1, :],
                                start=True, stop=True, tile_position=(0, 0),
                                skip_group_check=True,
                            )
                            nc.tensor.matmul(
                                scg[:], khT_sb[DK:P, 1, ksl], qhT_cur[DK:P, 1, :],
                                start=True, stop=True, tile_position=(DK, 0),
                                skip_group_check=True,
                            )
                            pt3 = sp.tile([P, 3, QB], f16, tag="pt3", bufs=3,
                                          name="pt3")
                            ptg = sp.tile([P, QB], f16, tag="ptg", bufs=3,
                                          name="ptg")
                            nc.vector.tensor_tensor(
                                pt3[:], scd[:], _bcast_mid(A_sb[:, i, :], 3),
                                mybir.AluOpType.mult,
                            )
                            sc_sb = sp.tile([P, QB], f32, tag="scb", bufs=6,
                                            name="sc_sb")
                            nc.scalar.copy(sc_sb[:], scg[:])
                            nc.gpsimd.tensor_tensor(
                                ptg[:], sc_sb[:], A_sb[:, i, :],
                                mybir.AluOpType.mult,
                            )
                            pts = [pt3[:, 0, :], pt3[:, 1, :], pt3[:, 2, :], ptg[:]]
                        else:
                            # scores: both heads of a pair run concurrently in
                            # the upper/lower PE row-quadrants
                            scs = []
                            for j in range(NPAIR):
                                sc_e = psU.tile([P, QB], f32, tag="sc", bufs=SC_BUFS, name="sc_e")
                                nc.tensor.matmul(
                                    sc_e[:], khT_sb[0:DK, j, ksl], qhT_cur[0:DK, j, :],
                                    start=True, stop=True, tile_position=(0, 0),
                                )
                                sc_o = psU.tile([P, QB], f32, tag="sc", bufs=SC_BUFS, name="sc_o")
                                nc.tensor.matmul(
                                    sc_o[:], khT_sb[DK:P, j, ksl], qhT_cur[DK:P, j, :],
                                    start=True, stop=True, tile_position=(DK, 0),
                                )
                                scs += [sc_e, sc_o]
                            # mask multiply, spread over DVE / (ScalarE+GPSIMD):
                            # heads 0,1 on DVE; heads 2,3 alternate by key-block
                            if PV_POS and len(pend_pts) >= PV_DEPTH:
                                emit_xt(*pend_pts.pop(0))
                            pts = []
                            for h in range(HPC):
                                pt = sp.tile([P, QB], f16, tag="pt", bufs=PT_BUFS, name="pt")
                                if MASK_ROUTE:
                                    r = MASK_ROUTE[(kb * HPC + h) % len(MASK_ROUTE)] \
                                        if len(MASK_ROUTE) > HPC else MASK_ROUTE[h]
                                    if QB0_PLAIN and qb == 0 and r == "a":
                                        r = "d"
                                    use_gp = r == "g"
                                    use_assist = r == "a"
                                else:
                                    gmod = GP_MOD1 if (GP_MOD1 and qb > 0) else GP_MOD
                                    use_gp = h == 3 or (h == 2 and kb % gmod == 0)
                                    use_assist = ACT_ASSIST and h == 2 and not use_gp
                                if use_assist:
                                    # ScalarE casts scores to f16 in SBUF, DVE
                                    # multiplies in 4x all-SBUF mode
                                    sc_sb = sp.tile([P, QB], f16, tag="scbh", bufs=4,
                                                    name="sc_sbh")
                                    nc.scalar.copy(sc_sb[:], scs[h][:])
                                    nc.vector.tensor_tensor(
                                        pt[:], sc_sb[:], A_sb[:, i, :],
                                        mybir.AluOpType.mult,
                                    )
                                elif use_gp and GP_PSUM:
                                    nc.gpsimd.tensor_tensor(
                                        pt[:], scs[h][:], A_sb[:, i, :],
                                        mybir.AluOpType.mult,
                                    )
                                elif use_gp:
                                    sc_sb = sp.tile([P, QB], f32, tag="scb", bufs=SCB_BUFS,
                                                    name="sc_sb")
                                    if BOUNCE_HALF:
                                        for hf in range(2):
                                            fsl = slice(hf * (QB // 2), (hf + 1) * (QB // 2))
                                            nc.scalar.copy(sc_sb[:, fsl], scs[h][:, fsl])
                                            nc.gpsimd.tensor_tensor(
                                                pt[:, fsl], sc_sb[:, fsl], A_sb[:, i, fsl],
                                                mybir.AluOpType.mult,
                                            )
                                    else:
                                        nc.scalar.copy(sc_sb[:], scs[h][:])
                                        nc.gpsimd.tensor_tensor(
                                            pt[:], sc_sb[:], A_sb[:, i, :],
                                            mybir.AluOpType.mult,
                                        )
                                else:
                                    nc.vector.tensor_tensor(
                                        pt[:], scs[h][:], A_sb[:, i, :],
                                        mybir.AluOpType.mult,
                                    )
                                pts.append(pt)
                        # software pipeline: emit an older key block's p@v
                        # matmuls now, so the PE never waits mid-iteration for
                        # this kb's mask mults
                        pend_pts.append((pts, kb))
                        if not PV_POS and len(pend_pts) >= PV_DEPTH:
                            emit_xt(*pend_pts.pop(0))
                        for _ in range(FILL_RATE):
                            if fillers:
                                fillers.pop(0)()


                while fillers:
                    fillers.pop(0)()
                while pend_pts:
                    emit_xt(*pend_pts.pop(0))  # drain the remaining key blocks
                pend_xts = (xts, qb)
                qhT_cur, qhT_next = qhT_next, None

            # drain the last query block's output projection through the
            # score banks (idle by now) for deeper tail pipelining
            xts_p, qb_p = pend_xts
            ep_tag, ep_bufs = ("u", U_BUFS) if (DVE_MERGE or SC_PAIR) \
                else ("sc", SC_BUFS)
            for ssub in range(QB // P):
                o_proj_chain(xts_p, qb_p, ssub, ptag=ep_tag, pbufs=ep_bufs,
                             epi=True)

    nc.compile()
    return nc


def _numpy_fallback(q, k, v, A, Wq, bq, Wk, bk, Wv, bv, Wo, bo):
    def proj(x, W, b):
        y = x @ W.T + b
        return y.reshape(B, S, H, DK).transpose(0, 2, 1, 3)

    qh, kh, vh = proj(q, Wq, bq), proj(k, Wk, bk), proj(v, Wv, bv)
    scores = np.einsum("bhqd,bhkd->bhqk", qh, kh) * np.float32(SCALE)
    p = scores * A.T
    x = np.einsum("bhqk,bhkd->bhqd", p, vh)
    x = x.transpose(0, 2, 1, 3).reshape(B, S, D)
    return (x @ Wo.T + bo).astype(np.float32)


def kernel(**inputs):
    q = np.asarray(inputs["q"], dtype=np.float32)
    k = np.asarray(inputs["k"], dtype=np.float32)
    v = np.asarray(inputs["v"], dtype=np.float32)
    A = np.asarray(inputs["A"], dtype=np.float32)
    Wq = np.asarray(inputs["Wq"], dtype=np.float32)
    Wk = np.asarray(inputs["Wk"], dtype=np.float32)
    Wv = np.asarray(inputs["Wv"], dtype=np.float32)
    Wo = np.asarray(inputs["Wo"], dtype=np.float32)
    bq, bk, bv, bo = (np.asarray(inputs[n], dtype=np.float32) for n in ("bq", "bk", "bv", "bo"))

    # The device kernel folds zero biases away (spec fills them with zeros);
    # fall back to a host reference in the (unused) nonzero-bias case.
    if any(np.any(b) for b in (bq, bk, bv)):
        return _numpy_fallback(q, k, v, A, Wq, bq, Wk, bk, Wv, bv, Wo, bo)

    global _CACHED
    if _CACHED is None:
        _CACHED = _build()
    nc = _CACHED

    Asc = np.ascontiguousarray((A * np.float32(SCALE)).astype(np.float16))
    in_maps = []
    for c in range(NCORES):
        b, g = divmod(c, GROUPS)
        hsl = slice(g * HD, (g + 1) * HD)
        in_maps.append({
            "qT": np.ascontiguousarray(q[b].T.astype(np.float16)),
            "kT": np.ascontiguousarray(k[b].T.astype(np.float16)),
            "vT": np.ascontiguousarray(v[b].T.astype(np.float16)),
            "Asc": Asc,
            "wq": np.ascontiguousarray(Wq[hsl].T.astype(np.float16)),
            "wk": np.ascontiguousarray(Wk[hsl].T.astype(np.float16)),
            "wv": np.ascontiguousarray(Wv[hsl].T.astype(np.float16)),
            "wo": np.ascontiguousarray(
                Wo[:, hsl].T.astype(np.float16) if WO_F16 else Wo[:, hsl].T),
        })

    res = bass_utils.run_bass_kernel_spmd(
        nc, in_maps, core_ids=list(range(NCORES)), trace=TRACE
    )
    global LAST_RESULTS
    LAST_RESULTS = res

    out = np.zeros((B, S, D), dtype=np.float32)
    for c in range(NCORES):
        out[c // GROUPS] += res.results[c]["out"].astype(np.float32)
    out += bo
    return out


if __name__ == "__main__":
    rng = np.random.default_rng(0)
    ins = {
        "q": rng.standard_normal((B, S, D), dtype=np.float32),
        "k": rng.standard_normal((B, S, D), dtype=np.float32),
        "v": rng.standard_normal((B, S, D), dtype=np.float32),
        "A": rng.random((S, S), dtype=np.float32),
        "Wq": rng.standard_normal((D, D), dtype=np.float32) / 32,
        "bq": np.zeros(D, np.float32),
        "Wk": rng.standard_normal((D, D), dtype=np.float32) / 32,
        "bk": np.zeros(D, np.float32),
        "Wv": rng.standard_normal((D, D), dtype=np.float32) / 32,
        "bv": np.zeros(D, np.float32),
        "Wo": rng.standard_normal((D, D), dtype=np.float32) / 32,
        "bo": np.zeros(D, np.float32),
    }
    got = kernel(**ins)
    ref = _numpy_fallback(**ins)
    err = np.abs(got - ref).max() / np.abs(ref).max()
    print("self-check relmax:", err)



# revision 15
# speedup vs baseline: 1.0418x; 1.0029x over previous
"""Trainium2 Bass kernel for nn_AttentionBlock (sparse_attention, no-softmax).

Computation (per batch b):
    qh = (q @ Wq^T) split into 16 heads of dk=64     [S, D] -> [H, S, DK]
    kh, vh likewise
    scores = (qh @ kh^T) / sqrt(DK)                  [H, S, S]
    p      = scores * A^T                            (elementwise structural mask)
    x      = p @ vh                                  [H, S, DK] -> [S, D]
    out    = x @ Wo^T + bo                           [S, D]

Sharding over 8 NeuronCores: data-parallel over batch (B=2) x tensor-parallel
over heads (16 heads -> 4 per core). Each core projects q/k/v for its 4 heads
(column-parallel), runs masked attention for them, and applies its 256-column
slice of the output projection (row-parallel), producing a full-shape partial
output. Host sums the 4 partials per batch.

Implementation notes:
- Activations are shipped pre-transposed ([D, S]) so every matmul contraction
  dim lands on SBUF partitions with no on-device transposes; 1/sqrt(DK) is
  folded into the mask A on the host.
- The whole data path runs in fp16 with fp32 PSUM accumulation (all operands
  here are O(1)-O(100), well inside fp16 range; measured end-to-end error is
  a few 1e-4). fp16 is the same PE stream rate as bf16/f32r but, being
  2-byte, additionally halves DMA/SBUF traffic and legalizes PE quadrant
  packing (tile_position), which f32/f32r reject.
- Heads are stored as pairs on the partition axis (head 2j on partitions
  0:63, head 2j+1 on 64:127). The K=64 score matmuls of a pair run
  concurrently in the upper/lower PE row-quadrants (tile_position (0,0) /
  (64,0)); the M=64 p@v matmuls of a pair run concurrently in left/right
  col-quadrants into one PSUM bank (tile_position (0,0) / (0,64)).
- The mask multiply is the throughput-critical elementwise stage; it is
  spread over three engines per the MASK_ROUTE pattern: DVE straight out of
  PSUM (most tiles), a ScalarE PSUM->SBUF bounce feeding GPSIMD (head 3 +
  head 2 every 8th key block), and occasionally a ScalarE f16 bounce feeding
  a DVE 2x all-SBUF multiply. GPSIMD cannot read PSUM (neuronxcc rejects
  it), so its tiles must bounce through ScalarE.
- Projection work for the next/previous query block is interleaved into the
  attention loop so no engine drains the pipeline at block boundaries.
  The prologue orders DMAs by true need time (k weights/activations, then
  q, then v, then the mask tiles); slice-0 v-chains drain as group-0
  fillers so scores start ~13us in. Partial outputs and the o-projection
  weights are fp16 (halves outbound DMA; host accumulates in fp32), and
  the epilogue alternates its PSUM->SBUF drains between ScalarE and DVE.
"""

import numpy as np

import concourse.mybir as mybir
import concourse.tile as tile
from concourse import bacc, bass_utils
from concourse.bass import AP


def _bcast_mid(ap2, n):
    """[128, F] AP -> [128, (0-stride n), F]: broadcast over an inserted
    middle dim so one tensor_tensor applies the same mask row-block to n
    head slots."""
    lay = [list(d) for d in ap2.ap]
    assert len(lay) == 2
    return AP(ap2.tensor, ap2.offset, [lay[0], [0, n], lay[1]])

B, S, D, H = 2, 2048, 1024, 16
NCORES = 8
GROUPS = NCORES // B          # 4 head-groups
HPC = H // GROUPS             # 4 heads per core
DK = D // H                   # 64
HD = HPC * DK                 # 256 head-dim columns per core
NPAIR = HPC // 2              # 2 head pairs per core
SCALE = 1.0 / np.sqrt(DK)

P = 128                       # SBUF partitions
QB = 512                      # query block
NQB = S // QB                 # 4
KBLK = 128                    # key block
NKB = S // KBLK               # 16
NKT = D // P                  # 8 contraction chunks for projections
AGRP = 4                      # key-blocks per A-tile DMA / interleave group
NGRP = NKB // AGRP            # 4 groups

f32 = mybir.dt.float32
f16 = mybir.dt.float16

import os
KV_SPLIT = int(os.environ.get("KV_SPLIT", "1"))       # split k/q chains in half
MERGE_KQ = int(os.environ.get("MERGE_KQ", "1"))       # merged khT/qhT drains
MERGE_XTS = int(os.environ.get("MERGE_XTS", "0"))     # merged xts drain
OPROJ_SPLIT = int(os.environ.get("OPROJ_SPLIT", "1")) # o-proj per-et thunks
A_PREF = int(os.environ.get("A_PREF", "1"))           # A prefetch depth (1|2)
GP_MOD = int(os.environ.get("GP_MOD", "8"))           # head2 on GP when kb%GP_MOD==0
OUT_F16 = int(os.environ.get("OUT_F16", "1"))         # fp16 partial outputs
DVE_MERGE = int(os.environ.get("DVE_MERGE", "0"))     # heads 0-2 in one DVE mask op
SCG_BUFS = int(os.environ.get("SCG_BUFS", "1"))       # psum bufs for the GP head's scores
U_BUFS = int(os.environ.get("U_BUFS", "2"))           # psum bufs for proj chains
SC_BUFS = int(os.environ.get("SC_BUFS", "4"))         # psum bufs for score tiles
BOUNCE_HALF = int(os.environ.get("BOUNCE_HALF", "0")) # split ScalarE->GPSIMD path in halves
GP_PSUM = int(os.environ.get("GP_PSUM", "0"))         # GPSIMD reads scores straight from PSUM
OPROJ_LATE = int(os.environ.get("OPROJ_LATE", "0"))   # last qb: prev o-proj in late groups
ACT_ASSIST = int(os.environ.get("ACT_ASSIST", "0"))   # h2 via Act f16-bounce + DVE 4x mult
MASK_ROUTE = os.environ.get("MASK_ROUTE", "dddgdddgdgdddgdadgggdddgddggddagddagdaggddgdddggdddagdagddagdada")         # per-head route: d=DVE, a=Act+DVE2x, g=GPSIMD
WO_F16 = int(os.environ.get("WO_F16", "1"))           # f16 output projection weights + xts
WO_DEFER = int(os.environ.get("WO_DEFER", "1"))       # load wo during qb0 (not prologue)
EPI_DVE = int(os.environ.get("EPI_DVE", "1"))         # epilogue osb copies alternate DVE/Act
GP_MOD1 = int(os.environ.get("GP_MOD1", "0"))         # GP_MOD override for qb>=1 (0=same)
EPI_DMA_SPLIT = int(os.environ.get("EPI_DMA_SPLIT", "0"))  # epilogue DMA per 512-col chunk
PRO_QFIRST = int(os.environ.get("PRO_QFIRST", "1"))   # prologue: q proj before v proj
V_FILLER = int(os.environ.get("V_FILLER", "0"))       # slice-0 v chains as group-0 fillers
A_EARLY = int(os.environ.get("A_EARLY", "0"))
A_AFTER_KV = int(os.environ.get("A_AFTER_KV", "1"))   # group A-prefetch after kv DMAs         # A0/A1 before the v stream
PT_BUFS = int(os.environ.get("PT_BUFS", "32"))        # sbuf bufs for mask outputs
SCB_BUFS = int(os.environ.get("SCB_BUFS", "12"))       # sbuf bufs for GP bounce tiles
SC_PAIR = int(os.environ.get("SC_PAIR", "0"))         # heads 0+1 share a 2-bank score tile
PRO_SPLIT2 = int(os.environ.get("PRO_SPLIT2", "0"))   # finer first wk/kT0 DMA chunks
QB0_PLAIN = int(os.environ.get("QB0_PLAIN", "0"))     # no Act-assist routing in qb0
PV_DEPTH = int(os.environ.get("PV_DEPTH", "7"))       # p@v software-pipeline depth (key blocks)
QT_SPLIT = int(os.environ.get("QT_SPLIT", "1"))       # split qT loads in kt halves
FILL_RATE = int(os.environ.get("FILL_RATE", "2"))     # filler thunks drained per key block
PV_POS = int(os.environ.get("PV_POS", "0"))           # emit pipelined p@v before the masks
WARMUP = int(os.environ.get("WARMUP", "0"))           # dummy matmul at t~0 to start PE p-state ramp
PRO_V2 = int(os.environ.get("PRO_V2", "0"))           # interleaved k/q prologue DMA + chain order
XTS_DVE = int(os.environ.get("XTS_DVE", "0"))         # xts drains: 0=Act,1=DVE,2=split
QDRAIN_DVE = int(os.environ.get("QDRAIN_DVE", "0"))   # qhT drains on DVE
OSB_DVE = int(os.environ.get("OSB_DVE", "0"))         # mid-kernel osb copies: DVE every Nth et
OPROJ_SPREAD = int(os.environ.get("OPROJ_SPREAD", "1"))  # 1 o-proj chain per group (not 2+2)
                                                      # and one merged DVE mask multiply
                                                      # e.g. "daag"; empty = legacy GP_MOD/ACT_ASSIST

_CACHED = None  # built module, reused across kernel() calls
TRACE = False         # set True (e.g. from test.py) to profile the NEFF
LAST_RESULTS = None   # BassKernelResults of the most recent run


def _build():
    nc = bacc.Bacc("TRN2", target_bir_lowering=False)

    qT = nc.dram_tensor("qT", [D, S], f16, kind="ExternalInput")
    kT = nc.dram_tensor("kT", [D, S], f16, kind="ExternalInput")
    vT = nc.dram_tensor("vT", [D, S], f16, kind="ExternalInput")
    Asc = nc.dram_tensor("Asc", [S, S], f16, kind="ExternalInput")
    wq = nc.dram_tensor("wq", [D, HD], f16, kind="ExternalInput")
    wk = nc.dram_tensor("wk", [D, HD], f16, kind="ExternalInput")
    wv = nc.dram_tensor("wv", [D, HD], f16, kind="ExternalInput")
    wo = nc.dram_tensor("wo", [HD, D],
                        f16 if WO_F16 else mybir.dt.float32r, kind="ExternalInput")
    # fp16 partial outputs: halves the outbound DMA; the host accumulates the
    # four per-batch partials in fp32 (adds ~1e-4 relative error)
    out = nc.dram_tensor("out", [S, D], f16 if OUT_F16 else f32, kind="ExternalOutput")

    qT_r = qT.rearrange("(kt p) s -> p kt s", p=P)
    kT_r = kT.rearrange("(kt p) s -> p kt s", p=P)
    vT_r = vT.rearrange("(kt p) s -> p kt s", p=P)
    wq_r = wq.rearrange("(kt p) c -> p kt c", p=P)
    wk_r = wk.rearrange("(kt p) c -> p kt c", p=P)
    wv_r = wv.rearrange("(kt p) c -> p kt c", p=P)
    wo_r = wo.rearrange("(ck p) e -> p ck e", p=P)
    A_r = Asc.rearrange("(kb p) q -> p kb q", p=P)

    with tile.TileContext(nc) as tc:
        with (
            tc.tile_pool(name="persist", bufs=1) as pp,
            tc.tile_pool(name="stream", bufs=2) as sp,
            tc.tile_pool(name="psU", bufs=2, space="PSUM") as psU,   # proj [128,512] x2 + sc x4
            tc.tile_pool(name="psX", bufs=1, space="PSUM") as psX,   # xT accumulators
        ):
            if WARMUP:
                # touch the PE immediately: pe_busy_start is sticky, so one
                # tiny matmul at t~0 means the p-state ramp competes with the
                # prologue DMAs instead of the first real chains
                wu_sb = pp.tile([P, 16], f16, tag="warm")
                nc.vector.memset(wu_sb[:], 0.0)
                wu_ps = psU.tile([P, 16], f32, tag="u", bufs=U_BUFS, name="warm")
                nc.tensor.matmul(wu_ps[0:16, :], wu_sb[:], wu_sb[:],
                                 start=True, stop=True)

            wk_sb = pp.tile([P, NKT, HD], f16, tag="wk")
            wv_sb = pp.tile([P, NKT, HD], f16, tag="wv")
            wq_sb = pp.tile([P, NKT, HD], f16, tag="wq")
            wo_sb = pp.tile([P, HD // P, D],
                            f16 if WO_F16 else mybir.dt.float32r, tag="wo")

            # head-PAIR layout: pair j holds head 2j on partitions 0:64 and
            # head 2j+1 on 64:128 — the layout quadrant packing requires
            khT_sb = pp.tile([P, NPAIR, S], f16, tag="khT")
            vh_sb = pp.tile([P, NKB, HD], f16, tag="vh")     # [ks%128, kb, c]

            def kv_proj_chains(st, kT_pre=None, vT_pre=None):
                """Issue k/v DMAs for s-slice st now; return per-chain thunks
                so the projection matmuls can be sprinkled between attention
                iterations instead of lumped at group boundaries."""
                sl = slice(st * QB, (st + 1) * QB)
                kT_sb = kT_pre
                if kT_sb is None:
                    kT_sb = sp.tile([P, NKT, QB], f16, tag="xin", bufs=4, name="kT_sb")
                    nc.sync.dma_start(kT_sb[:], kT_r[:, :, sl])
                vT_sb = vT_pre
                if vT_sb is None:
                    vT_sb = sp.tile([P, NKT, QB], f16, tag="xin", bufs=4, name="vT_sb")
                    nc.sync.dma_start(vT_sb[:], vT_r[:, :, sl])

                pk_tiles = {}

                def kchain_a(ct, kT_sb=kT_sb):
                    pk = psU.tile([P, QB], f32, tag="u", bufs=U_BUFS, name="pk")
                    pk_tiles[ct] = pk
                    hi = NKT // 2 if KV_SPLIT else NKT
                    for kt in range(hi):
                        nc.tensor.matmul(
                            pk[:], wk_sb[:, kt, ct * P:(ct + 1) * P], kT_sb[:, kt, :],
                            start=(kt == 0), stop=(kt == NKT - 1),
                        )
                    if not KV_SPLIT:
                        kdrain(ct)

                def kdrain(ct):
                    pk = pk_tiles.pop(ct)
                    if MERGE_KQ:
                        nc.scalar.copy(khT_sb[:, ct, sl], pk[:])
                    else:
                        nc.scalar.copy(khT_sb[0:DK, ct, sl], pk[0:DK, :])
                        nc.scalar.copy(khT_sb[DK:P, ct, sl], pk[DK:P, :])

                def kchain_b(ct, kT_sb=kT_sb):
                    pk = pk_tiles[ct]
                    for kt in range(NKT // 2, NKT):
                        nc.tensor.matmul(
                            pk[:], wk_sb[:, kt, ct * P:(ct + 1) * P], kT_sb[:, kt, :],
                            start=False, stop=(kt == NKT - 1),
                        )
                    kdrain(ct)

                def vchain(ssub, vT_sb=vT_sb):
                    kb = st * (QB // P) + ssub
                    pv = psU.tile([P, HD], f32, tag="u", bufs=U_BUFS, name="pv")
                    for kt in range(NKT):
                        nc.tensor.matmul(
                            pv[:], vT_sb[:, kt, ssub * P:(ssub + 1) * P], wv_sb[:, kt, :],
                            start=(kt == 0), stop=(kt == NKT - 1),
                        )
                    nc.scalar.copy(vh_sb[:, kb, :], pv[:])

                thunks = []
                for ct in range(NPAIR):
                    thunks.append(lambda ct=ct: kchain_a(ct))
                    if KV_SPLIT:
                        thunks.append(lambda ct=ct: kchain_b(ct))
                thunks += [lambda s=s: vchain(s) for s in range(QB // P)]
                return thunks

            def kv_proj(st, kT_pre=None, vT_pre=None):
                for thunk in kv_proj_chains(st, kT_pre, vT_pre):
                    thunk()

            def q_proj_chains(qb, qT_pre=None):
                """Issue the q DMA now; return (qhT tile, per-pair chain thunks)."""
                qsl = slice(qb * QB, (qb + 1) * QB)
                qT_sb = qT_pre
                if qT_sb is None:
                    qT_sb = sp.tile([P, NKT, QB], f16, tag="xin", bufs=4, name="qT_sb")
                    if QT_SPLIT:
                        nc.sync.dma_start(qT_sb[:, 0:NKT // 2, :], qT_r[:, 0:NKT // 2, qsl])
                        nc.sync.dma_start(qT_sb[:, NKT // 2:, :], qT_r[:, NKT // 2:, qsl])
                    else:
                        nc.sync.dma_start(qT_sb[:], qT_r[:, :, qsl])
                qhT_sb = sp.tile([P, NPAIR, QB], f16, tag="qh", bufs=3, name="qhT_sb")

                pq_tiles = {}

                def chain_a(ct):
                    pq = psU.tile([P, QB], f32, tag="u", bufs=U_BUFS, name="pq")
                    pq_tiles[ct] = pq
                    hi = NKT // 2 if KV_SPLIT else NKT
                    for kt in range(hi):
                        nc.tensor.matmul(
                            pq[:], wq_sb[:, kt, ct * P:(ct + 1) * P], qT_sb[:, kt, :],
                            start=(kt == 0), stop=(kt == NKT - 1),
                        )
                    if not KV_SPLIT:
                        qdrain(ct)

                def qdrain(ct):
                    pq = pq_tiles.pop(ct)
                    if QDRAIN_DVE:
                        nc.vector.tensor_copy(qhT_sb[:, ct, :], pq[:])
                    elif MERGE_KQ:
                        nc.scalar.copy(qhT_sb[:, ct, :], pq[:])
                    else:
                        nc.scalar.copy(qhT_sb[0:DK, ct, :], pq[0:DK, :])
                        nc.scalar.copy(qhT_sb[DK:P, ct, :], pq[DK:P, :])

                def chain_b(ct):
                    pq = pq_tiles[ct]
                    for kt in range(NKT // 2, NKT):
                        nc.tensor.matmul(
                            pq[:], wq_sb[:, kt, ct * P:(ct + 1) * P], qT_sb[:, kt, :],
                            start=False, stop=(kt == NKT - 1),
                        )
                    qdrain(ct)

                thunks = []
                for ct in range(NPAIR):
                    thunks.append(lambda ct=ct: chain_a(ct))
                    if KV_SPLIT:
                        thunks.append(lambda ct=ct: chain_b(ct))
                return qhT_sb, thunks

            def q_proj(qb):
                qhT_sb, chains = q_proj_chains(qb)
                for thunk in chains:
                    thunk()
                return qhT_sb

            def o_proj_et(xts, qb, ssub, et, osb, ptag="u", pbufs=None,
                          dve_copy=False, epi_split=False):
                """One 512-col chunk of the output projection for query block
                qb, row-slice ssub; et==last also issues the out DMA."""
                if pbufs is None and ptag == "u":
                    pbufs = U_BUFS
                po = psU.tile([P, QB], f32, tag=ptag, bufs=pbufs, name="po")
                for ck in range(HD // P):
                    nc.tensor.matmul(
                        po[:],
                        xts[:, ck, ssub * P:(ssub + 1) * P],
                        wo_sb[:, ck, et * QB:(et + 1) * QB],
                        start=(ck == 0), stop=(ck == HD // P - 1),
                    )
                if dve_copy or (OSB_DVE and et % OSB_DVE == 0):
                    nc.vector.tensor_copy(osb[:, et * QB:(et + 1) * QB], po[:])
                else:
                    nc.scalar.copy(osb[:, et * QB:(et + 1) * QB], po[:])
                rsl = slice(qb * QB + ssub * P, qb * QB + (ssub + 1) * P)
                if epi_split:
                    nc.sync.dma_start(
                        out[rsl, et * QB:(et + 1) * QB],
                        osb[:, et * QB:(et + 1) * QB],
                    )
                elif et == D // QB - 1:
                    nc.sync.dma_start(out[rsl, :], osb[:])

            def o_proj_chain_thunks(xts, qb, ssub, ptag="u", pbufs=None,
                                    epi=False):
                osb = sp.tile([P, D], f16 if OUT_F16 else f32, tag="osb", bufs=4, name="osb")
                ets = [
                    lambda et=et: o_proj_et(xts, qb, ssub, et, osb, ptag, pbufs,
                                            dve_copy=epi and EPI_DVE and et % 2 == 0,
                                            epi_split=epi and EPI_DMA_SPLIT)
                    for et in range(D // QB)
                ]
                if OPROJ_SPLIT:
                    return ets
                def whole():
                    for t in ets:
                        t()
                return [whole]

            def o_proj_chain(xts, qb, ssub, ptag="u", pbufs=None, epi=False):
                for thunk in o_proj_chain_thunks(xts, qb, ssub, ptag, pbufs, epi):
                    thunk()

            # ---- pipeline ---------------------------------------------------------
            # prologue DMAs in dependency-first order so the PE starts ASAP
            kT0 = sp.tile([P, NKT, QB], f16, tag="xin", bufs=4, name="kT_sb")
            if PRO_V2:
                # interleave the k and q projection DMA streams so both paths'
                # chain_a/chain_b pairs consume data as it lands; the first
                # score matmul is gated by the LAST proj drain, so balancing
                # the two DMA paths beats finishing one path first
                H2 = NKT // 2
                qT0 = sp.tile([P, NKT, QB], f16, tag="xin", bufs=4, name="qT_sb")
                vT0 = sp.tile([P, NKT, QB], f16, tag="xin", bufs=4, name="vT_sb")
                nc.sync.dma_start(wk_sb[:, 0:H2, :], wk_r[:, 0:H2, :])
                nc.sync.dma_start(kT0[:, 0:H2, :], kT_r[:, 0:H2, 0:QB])
                nc.sync.dma_start(wq_sb[:, 0:H2, :], wq_r[:, 0:H2, :])
                nc.sync.dma_start(qT0[:, 0:H2, :], qT_r[:, 0:H2, 0:QB])
                nc.sync.dma_start(wk_sb[:, H2:, :], wk_r[:, H2:, :])
                nc.sync.dma_start(kT0[:, H2:, :], kT_r[:, H2:, 0:QB])
                nc.sync.dma_start(wq_sb[:, H2:, :], wq_r[:, H2:, :])
                nc.sync.dma_start(qT0[:, H2:, :], qT_r[:, H2:, 0:QB])
                fetch_A_early = True
            else:
                fetch_A_early = False
            if not PRO_V2:
                if PRO_SPLIT2:
                    nc.sync.dma_start(wk_sb[:, 0:2, :], wk_r[:, 0:2, :])
                    nc.sync.dma_start(kT0[:, 0:2, :], kT_r[:, 0:2, 0:QB])
                    nc.sync.dma_start(wk_sb[:, 2:NKT // 2, :], wk_r[:, 2:NKT // 2, :])
                    nc.sync.dma_start(kT0[:, 2:NKT // 2, :], kT_r[:, 2:NKT // 2, 0:QB])
                else:
                    nc.sync.dma_start(wk_sb[:, 0:NKT // 2, :], wk_r[:, 0:NKT // 2, :])
                    nc.sync.dma_start(kT0[:, 0:NKT // 2, :], kT_r[:, 0:NKT // 2, 0:QB])
                nc.sync.dma_start(wk_sb[:, NKT // 2:, :], wk_r[:, NKT // 2:, :])
                nc.sync.dma_start(kT0[:, NKT // 2:, :], kT_r[:, NKT // 2:, 0:QB])
                if not PRO_QFIRST:
                    nc.sync.dma_start(wv_sb[:], wv_r[:])
                vT0 = sp.tile([P, NKT, QB], f16, tag="xin", bufs=4, name="vT_sb")
                if not PRO_QFIRST:
                    nc.sync.dma_start(vT0[:], vT_r[:, :, 0:QB])
                nc.sync.dma_start(wq_sb[:], wq_r[:])

            # A tiles are fetched two groups ahead; group sequence is linear
            # over (qb, kbg)
            gseq = [(qb, kbg) for qb in range(NQB) for kbg in range(NGRP)]
            A_tiles = {}

            def fetch_A(gidx):
                if gidx >= len(gseq):
                    return
                aqb, akbg = gseq[gidx]
                A_sb = sp.tile([P, AGRP, QB], f16, tag="A", bufs=4, name="A_sb")
                nc.sync.dma_start(
                    A_sb[:],
                    A_r[:, akbg * AGRP:(akbg + 1) * AGRP, aqb * QB:(aqb + 1) * QB],
                )
                A_tiles[gidx] = A_sb

            pend_fillers = []
            if PRO_V2:
                # PE order: kchain_a pair, qchain_a pair (consume first DMA
                # halves), then kchain_b / qchain_b (+drains) on the second
                # halves; A and the v stream follow the proj DMAs
                qhT_cur, qchains0 = q_proj_chains(0, qT_pre=qT0)
                fetch_A(0)
                if A_PREF >= 2:
                    fetch_A(1)
                nc.sync.dma_start(wv_sb[:], wv_r[:])
                nc.sync.dma_start(vT0[:], vT_r[:, :, 0:QB])
                thunks0 = kv_proj_chains(0, kT_pre=kT0, vT_pre=vT0)
                assert KV_SPLIT, "PRO_V2 requires split chains"
                # interleave: ka0 ka1 | qa0 qa1 | kb0 kb1 | qb0 qb1
                for t in (thunks0[0], thunks0[2]):   # kchain_a ct0, ct1
                    t()
                for t in (qchains0[0], qchains0[2]):  # qchain_a ct0, ct1
                    t()
                for t in (thunks0[1], thunks0[3]):   # kchain_b ct0, ct1
                    t()
                for t in (qchains0[1], qchains0[3]):  # qchain_b ct0, ct1
                    t()
                if V_FILLER:
                    pend_fillers = thunks0[2 * NPAIR:]
                else:
                    for t in thunks0[2 * NPAIR:]:
                        t()
            elif PRO_QFIRST:
                # q projection (and its DMA) before the v stream: scores can
                # start ~6us earlier; v data is not needed until the first p@v
                qhT_cur, qchains0 = q_proj_chains(0)
                if A_EARLY:
                    fetch_A(0)
                    if A_PREF >= 2:
                        fetch_A(1)
                nc.sync.dma_start(wv_sb[:], wv_r[:])
                nc.sync.dma_start(vT0[:], vT_r[:, :, 0:QB])
                if not A_EARLY:
                    fetch_A(0)
                    if A_PREF >= 2:
                        fetch_A(1)
                thunks0 = kv_proj_chains(0, kT_pre=kT0, vT_pre=vT0)
                for t in thunks0[:2 * NPAIR]:   # k chains
                    t()
                for t in qchains0:              # q chains
                    t()
                if V_FILLER:
                    pend_fillers = thunks0[2 * NPAIR:]
                else:
                    for t in thunks0[2 * NPAIR:]:   # v chains
                        t()
            else:
                fetch_A(0)
                kv_proj(0, kT_pre=kT0, vT_pre=vT0)
                qhT_cur = q_proj(0)
                if A_PREF >= 2:
                    fetch_A(1)
            if not WO_DEFER:
                nc.sync.dma_start(wo_sb[:], wo_r[:])

            pend_xts = None    # (xts tile, qb) awaiting output projection
            qhT_next = None

            for qb in range(NQB):
                qsl = slice(qb * QB, (qb + 1) * QB)
                xt = psX.tile([P, NPAIR, QB], f32, tag="xt", name="xt")  # 2 banks
                xts = sp.tile([P, NPAIR, QB],
                              f16 if WO_F16 else mybir.dt.float32r,
                              tag="xts", bufs=3, name="xts")
                pend_pts = []

                def emit_xt(pts, kb, xt=xt, xts=xts):
                    # p @ v: both heads of a pair run concurrently in the
                    # left/right PE col-quadrants into one PSUM bank.
                    # skip_group_check: the two col-quadrant groups legally
                    # share one PSUM bank (sim-only guard).
                    for j in range(NPAIR):
                        nc.tensor.matmul(
                            xt[0:DK, j, :],
                            vh_sb[:, kb, (2 * j) * DK:(2 * j + 1) * DK],
                            pts[2 * j],
                            start=(kb == 0), stop=(kb == NKB - 1),
                            tile_position=(0, 0), skip_group_check=True,
                        )
                        nc.tensor.matmul(
                            xt[DK:P, j, :],
                            vh_sb[:, kb, (2 * j + 1) * DK:(2 * j + 2) * DK],
                            pts[2 * j + 1],
                            start=(kb == 0), stop=(kb == NKB - 1),
                            tile_position=(0, DK), skip_group_check=True,
                        )
                    if kb == NKB - 1:
                        if MERGE_XTS:
                            nc.scalar.copy(xts[:], xt[:])
                        elif XTS_DVE == 1:
                            nc.vector.tensor_copy(xts[:, 0, :], xt[:, 0, :])
                            nc.vector.tensor_copy(xts[:, 1, :], xt[:, 1, :])
                        elif XTS_DVE == 2:
                            nc.scalar.copy(xts[:, 0, :], xt[:, 0, :])
                            nc.vector.tensor_copy(xts[:, 1, :], xt[:, 1, :])
                        else:
                            nc.scalar.copy(xts[:, 0, :], xt[:, 0, :])
                            nc.scalar.copy(xts[:, 1, :], xt[:, 1, :])

                fillers = list(pend_fillers)
                pend_fillers = []
                for kbg in range(NGRP):
                    gidx = qb * NGRP + kbg
                    if gidx not in A_tiles:
                        fetch_A(gidx)
                    A_sb = A_tiles.pop(gidx)
                    if not A_AFTER_KV:
                        fetch_A(gidx + A_PREF)
                    # queue this group's independent projection work; it is
                    # drained two chains per key-block below, keeping the PE
                    # fed without starving the elementwise engines
                    if qb == 0:
                        if kbg == 2 and WO_DEFER:
                            nc.sync.dma_start(wo_sb[:], wo_r[:])
                        if kbg < NGRP - 1:
                            fillers += kv_proj_chains(kbg + 1)
                        else:
                            qhT_next, qchains = q_proj_chains(1)
                            fillers += qchains
                    else:
                        late = OPROJ_LATE and qb == NQB - 1
                        if OPROJ_SPREAD:
                            og = kbg if not late else kbg - 1
                            lastog = NGRP - 1 if not late else NGRP - 2
                        else:
                            og = kbg - 2 if late else kbg
                            lastog = 1
                        if 0 <= og <= lastog and pend_xts is not None:
                            xts_p, qb_p = pend_xts
                            if OPROJ_SPREAD:
                                fillers += o_proj_chain_thunks(xts_p, qb_p, og)
                            else:
                                fillers += o_proj_chain_thunks(xts_p, qb_p, 2 * og)
                                fillers += o_proj_chain_thunks(xts_p, qb_p, 2 * og + 1)
                            if og == lastog:
                                pend_xts = None
                        if kbg == NGRP - 1 and qb < NQB - 1:
                            qhT_next, qchains = q_proj_chains(qb + 1)
                            fillers += qchains
                    if A_AFTER_KV:
                        fetch_A(gidx + A_PREF)
                    for i in range(AGRP):
                        kb = kbg * AGRP + i
                        ksl = slice(kb * KBLK, (kb + 1) * KBLK)
                        if SC_PAIR:
                            # heads 0,1: quadrant-packed score pair into one
                            # 2-bank tile; both banks finish together, so one
                            # merged DVE multiply adds no latency
                            sc2 = psU.tile([P, 2, QB], f32, tag="sc2", bufs=1,
                                           name="sc2")
                            sch2 = psU.tile([P, QB], f32, tag="sch2", bufs=1,
                                            name="sch2")
                            sch3 = psU.tile([P, QB], f32, tag="sch3", bufs=1,
                                            name="sch3")
                            nc.tensor.matmul(
                                sc2[:, 0, :], khT_sb[0:DK, 0, ksl], qhT_cur[0:DK, 0, :],
                                start=True, stop=True, tile_position=(0, 0),
                                skip_group_check=True,
                            )
                            nc.tensor.matmul(
                                sc2[:, 1, :], khT_sb[DK:P, 0, ksl], qhT_cur[DK:P, 0, :],
                                start=True, stop=True, tile_position=(DK, 0),
                                skip_group_check=True,
                            )
                            nc.tensor.matmul(
                                sch2[:], khT_sb[0:DK, 1, ksl], qhT_cur[0:DK, 1, :],
                                start=True, stop=True, tile_position=(0, 0),
                                skip_group_check=True,
                            )
                            nc.tensor.matmul(
                                sch3[:], khT_sb[DK:P, 1, ksl], qhT_cur[DK:P, 1, :],
                                start=True, stop=True, tile_position=(DK, 0),
                                skip_group_check=True,
                            )
                            pt2 = sp.tile([P, 2, QB], f16, tag="pt2", bufs=3,
                                          name="pt2")
                            nc.vector.tensor_tensor(
                                pt2[:], sc2[:], _bcast_mid(A_sb[:, i, :], 2),
                                mybir.AluOpType.mult,
                            )
                            pts = [pt2[:, 0, :], pt2[:, 1, :]]
                            for h, sch in ((2, sch2), (3, sch3)):
                                pt = sp.tile([P, QB], f16, tag="pt", bufs=PT_BUFS,
                                             name="pt")
                                r = MASK_ROUTE[(kb * HPC + h) % len(MASK_ROUTE)] \
                                    if len(MASK_ROUTE) > HPC else MASK_ROUTE[h]
                                if r == "g":
                                    sc_sb = sp.tile([P, QB], f32, tag="scb",
                                                    bufs=SCB_BUFS, name="sc_sb")
                                    nc.scalar.copy(sc_sb[:], sch[:])
                                    nc.gpsimd.tensor_tensor(
                                        pt[:], sc_sb[:], A_sb[:, i, :],
                                        mybir.AluOpType.mult,
                                    )
                                elif r == "a":
                                    sc_sb = sp.tile([P, QB], f16, tag="scbh", bufs=4,
                                                    name="sc_sbh")
                                    nc.scalar.copy(sc_sb[:], sch[:])
                                    nc.vector.tensor_tensor(
                                        pt[:], sc_sb[:], A_sb[:, i, :],
                                        mybir.AluOpType.mult,
                                    )
                                else:
                                    nc.vector.tensor_tensor(
                                        pt[:], sch[:], A_sb[:, i, :],
                                        mybir.AluOpType.mult,
                                    )
                                pts.append(pt)
                        elif DVE_MERGE:
                            # heads 0-2 share one 3-bank tile (single DVE
                            # consumer); head 3 gets its own bank for the
                            # ScalarE->GPSIMD path
                            scd = psU.tile([P, 3, QB], f32, tag="scd", bufs=1,
                                           name="scd")
                            scg = psU.tile([P, QB], f32, tag="scg",
                                           bufs=SCG_BUFS, name="scg")
                            nc.tensor.matmul(
                                scd[:, 0, :], khT_sb[0:DK, 0, ksl], qhT_cur[0:DK, 0, :],
                                start=True, stop=True, tile_position=(0, 0),
                                skip_group_check=True,
                            )
                            nc.tensor.matmul(
                                scd[:, 1, :], khT_sb[DK:P, 0, ksl], qhT_cur[DK:P, 0, :],
                                start=True, stop=True, tile_position=(DK, 0),
                                skip_group_check=True,
                            )
                            nc.tensor.matmul(
                                scd[:, 2, :], khT_sb[0:DK, 1, ksl], qhT_cur[0:DK, 1, :],
                                start=True, stop=True, tile_position=(0, 0),
                                skip_group_check=True,
                            )
                            nc.tensor.matmul(
                                scg[:], khT_sb[DK:P, 1, ksl], qhT_cur[DK:P, 1, :],
                                start=True, stop=True, tile_position=(DK, 0),
                                skip_group_check=True,
                            )
                            pt3 = sp.tile([P, 3, QB], f16, tag="pt3", bufs=3,
                                          name="pt3")
                            ptg = sp.tile([P, QB], f16, tag="ptg", bufs=3,
                                          name="ptg")
                            nc.vector.tensor_tensor(
                                pt3[:], scd[:], _bcast_mid(A_sb[:, i, :], 3),
                                mybir.AluOpType.mult,
                            )
                            sc_sb = sp.tile([P, QB], f32, tag="scb", bufs=6,
                                            name="sc_sb")
                            nc.scalar.copy(sc_sb[:], scg[:])
                            nc.gpsimd.tensor_tensor(
                                ptg[:], sc_sb[:], A_sb[:, i, :],
                                mybir.AluOpType.mult,
                            )
                            pts = [pt3[:, 0, :], pt3[:, 1, :], pt3[:, 2, :], ptg[:]]
                        else:
                            # scores: both heads of a pair run concurrently in
                            # the upper/lower PE row-quadrants
                            scs = []
                            for j in range(NPAIR):
                                sc_e = psU.tile([P, QB], f32, tag="sc", bufs=SC_BUFS, name="sc_e")
                                nc.tensor.matmul(
                                    sc_e[:], khT_sb[0:DK, j, ksl], qhT_cur[0:DK, j, :],
                                    start=True, stop=True, tile_position=(0, 0),
                                )
                                sc_o = psU.tile([P, QB], f32, tag="sc", bufs=SC_BUFS, name="sc_o")
                                nc.tensor.matmul(
                                    sc_o[:], khT_sb[DK:P, j, ksl], qhT_cur[DK:P, j, :],
                                    start=True, stop=True, tile_position=(DK, 0),
                                )
                                scs += [sc_e, sc_o]
                            # mask multiply, spread over DVE / (ScalarE+GPSIMD):
                            # heads 0,1 on DVE; heads 2,3 alternate by key-block
                            if PV_POS and len(pend_pts) >= PV_DEPTH:
                                emit_xt(*pend_pts.pop(0))
                            pts = []
                            for h in range(HPC):
                                pt = sp.tile([P, QB], f16, tag="pt", bufs=PT_BUFS, name="pt")
                                if MASK_ROUTE:
                                    r = MASK_ROUTE[(kb * HPC + h) % len(MASK_ROUTE)] \
                                        if len(MASK_ROUTE) > HPC else MASK_ROUTE[h]
                                    if QB0_PLAIN and qb == 0 and r == "a":
                                        r = "d"
                                    use_gp = r == "g"
                                    use_assist = r == "a"
                                else:
                                    gmod = GP_MOD1 if (GP_MOD1 and qb > 0) else GP_MOD
                                    use_gp = h == 3 or (h == 2 and kb % gmod == 0)
                                    use_assist = ACT_ASSIST and h == 2 and not use_gp
                                if use_assist:
                                    # ScalarE casts scores to f16 in SBUF, DVE
                                    # multiplies in 4x all-SBUF mode
                                    sc_sb = sp.tile([P, QB], f16, tag="scbh", bufs=4,
                                                    name="sc_sbh")
                                    nc.scalar.copy(sc_sb[:], scs[h][:])
                                    nc.vector.tensor_tensor(
                                        pt[:], sc_sb[:], A_sb[:, i, :],
                                        mybir.AluOpType.mult,
                                    )
                                elif use_gp and GP_PSUM:
                                    nc.gpsimd.tensor_tensor(
                                        pt[:], scs[h][:], A_sb[:, i, :],
                                        mybir.AluOpType.mult,
                                    )
                                elif use_gp:
                                    sc_sb = sp.tile([P, QB], f32, tag="scb", bufs=SCB_BUFS,
                                                    name="sc_sb")
                                    if BOUNCE_HALF:
                                        for hf in range(2):
                                            fsl = slice(hf * (QB // 2), (hf + 1) * (QB // 2))
                                            nc.scalar.copy(sc_sb[:, fsl], scs[h][:, fsl])
                                            nc.gpsimd.tensor_tensor(
                                                pt[:, fsl], sc_sb[:, fsl], A_sb[:, i, fsl],
                                                mybir.AluOpType.mult,
                                            )
                                    else:
                                        nc.scalar.copy(sc_sb[:], scs[h][:])
                                        nc.gpsimd.tensor_tensor(
                                            pt[:], sc_sb[:], A_sb[:, i, :],
                                            mybir.AluOpType.mult,
                                        )
                                else:
                                    nc.vector.tensor_tensor(
                                        pt[:], scs[h][:], A_sb[:, i, :],
                                        mybir.AluOpType.mult,
                                    )
                                pts.append(pt)
                        # software pipeline: emit an older key block's p@v
                        # matmuls now, so the PE never waits mid-iteration for
                        # this kb's mask mults
                        pend_pts.append((pts, kb))
                        if not PV_POS and len(pend_pts) >= PV_DEPTH:
                            emit_xt(*pend_pts.pop(0))
                        for _ in range(FILL_RATE):
                            if fillers:
                                fillers.pop(0)()


                while fillers:
                    fillers.pop(0)()
                while pend_pts:
                    emit_xt(*pend_pts.pop(0))  # drain the remaining key blocks
                pend_xts = (xts, qb)
                qhT_cur, qhT_next = qhT_next, None

            # drain the last query block's output projection through the
            # score banks (idle by now) for deeper tail pipelining
            xts_p, qb_p = pend_xts
            ep_tag, ep_bufs = ("u", U_BUFS) if (DVE_MERGE or SC_PAIR) \
                else ("sc", SC_BUFS)
            for ssub in range(QB // P):
                o_proj_chain(xts_p, qb_p, ssub, ptag=ep_tag, pbufs=ep_bufs,
                             epi=True)

    nc.compile()
    return nc


def _numpy_fallback(q, k, v, A, Wq, bq, Wk, bk, Wv, bv, Wo, bo):
    def proj(x, W, b):
        y = x @ W.T + b
        return y.reshape(B, S, H, DK).transpose(0, 2, 1, 3)

    qh, kh, vh = proj(q, Wq, bq), proj(k, Wk, bk), proj(v, Wv, bv)
    scores = np.einsum("bhqd,bhkd->bhqk", qh, kh) * np.float32(SCALE)
    p = scores * A.T
    x = np.einsum("bhqk,bhkd->bhqd", p, vh)
    x = x.transpose(0, 2, 1, 3).reshape(B, S, D)
    return (x @ Wo.T + bo).astype(np.float32)


def kernel(**inputs):
    q = np.asarray(inputs["q"], dtype=np.float32)
    k = np.asarray(inputs["k"], dtype=np.float32)
    v = np.asarray(inputs["v"], dtype=np.float32)
    A = np.asarray(inputs["A"], dtype=np.float32)
    Wq = np.asarray(inputs["Wq"], dtype=np.float32)
    Wk = np.asarray(inputs["Wk"], dtype=np.float32)
    Wv = np.asarray(inputs["Wv"], dtype=np.float32)
    Wo = np.asarray(inputs["Wo"], dtype=np.float32)
    bq, bk, bv, bo = (np.asarray(inputs[n], dtype=np.float32) for n in ("bq", "bk", "bv", "bo"))

    # The device kernel folds zero biases away (spec fills them with zeros);
    # fall back to a host reference in the (unused) nonzero-bias case.
    if any(np.any(b) for b in (bq, bk, bv)):
        return _numpy_fallback(q, k, v, A, Wq, bq, Wk, bk, Wv, bv, Wo, bo)

    global _CACHED
    if _CACHED is None:
        _CACHED = _build()
    nc = _CACHED

    Asc = np.ascontiguousarray((A * np.float32(SCALE)).astype(np.float16))
    in_maps = []
    for c in range(NCORES):
        b, g = divmod(c, GROUPS)
        hsl = slice(g * HD, (g + 1) * HD)
        in_maps.append({
            "qT": np.ascontiguousarray(q[b].T.astype(np.float16)),
            "kT": np.ascontiguousarray(k[b].T.astype(np.float16)),
            "vT": np.ascontiguousarray(v[b].T.astype(np.float16)),
            "Asc": Asc,
            "wq": np.ascontiguousarray(Wq[hsl].T.astype(np.float16)),
            "wk": np.ascontiguousarray(Wk[hsl].T.astype(np.float16)),
            "wv": np.ascontiguousarray(Wv[hsl].T.astype(np.float16)),
            "wo": np.ascontiguousarray(
                Wo[:, hsl].T.astype(np.float16) if WO_F16 else Wo[:, hsl].T),
        })

    res = bass_utils.run_bass_kernel_spmd(
        nc, in_maps, core_ids=list(range(NCORES)), trace=TRACE
    )
    global LAST_RESULTS
    LAST_RESULTS = res

    out = np.zeros((B, S, D), dtype=np.float32)
    for c in range(NCORES):
        out[c // GROUPS] += res.results[c]["out"].astype(np.float32)
    out += bo
    return out


if __name__ == "__main__":
    rng = np.random.default_rng(0)
    ins = {
        "q": rng.standard_normal((B, S, D), dtype=np.float32),
        "k": rng.standard_normal((B, S, D), dtype=np.float32),
        "v": rng.standard_normal((B, S, D), dtype=np.float32),
        "A": rng.random((S, S), dtype=np.float32),
        "Wq": rng.standard_normal((D, D), dtype=np.float32) / 32,
        "bq": np.zeros(D, np.float32),
        "Wk": rng.standard_normal((D, D), dtype=np.float32) / 32,
        "bk": np.zeros(D, np.float32),
        "Wv": rng.standard_normal((D, D), dtype=np.float32) / 32,
        "bv": np.zeros(D, np.float32),
        "Wo": rng.standard_normal((D, D), dtype=np.float32) / 32,
        "bo": np.zeros(D, np.float32),
    }
    got = kernel(**ins)
    ref = _numpy_fallback(**ins)
    err = np.abs(got - ref).max() / np.abs(ref).max()
    print("self-check relmax:", err)

